# revision 1
# baseline (speedup 1.0000x reference)
"""Bootstrap-ensemble MLP (100 models, D=16 -> H=128 x5 -> mu/sigma heads)
on 8 Trainium2 NeuronCores.

Sharding: every core runs an identical SPMD program over 25 models x 8192
batch points (model axis split 4 ways x batch split 2 ways) -- perfectly
balanced.  All per-core weights are pre-arranged on the host into the exact
SBUF layouts the TensorEngine wants (lhsT = pre-transposed stationary
operand), so the device does no transposes at all.

Compute structure per core:
- fp32r matmuls (TF32-class precision, 4x the throughput of fp32 on the PE)
- models interleaved in groups of 4 so PE always has independent matmuls
  while ACT/DVE run another model's bias+ReLU (fused into one op each)
- layer-1 (K=16) matmuls of the 4 models in a group run concurrently in
  different 32-row groups of the PE array (tile_position row tiling)
- mu/sigma head matmuls of model pairs run concurrently in different column
  halves (tile_position col tiling), accumulating all 25 models into one
  [128, CH] PSUM tile via zero-padded per-model head weights; finished with
  Identity/Exp activations with the bias folded in.
"""

import os

import numpy as np

M = 100  # n_models
D = 16  # input_dim
H = 128  # hidden_dim
O = 1  # output_dim
NH = 4  # n_hidden
N = 16384  # batch of query points

NCORES = 8
MPC = 25  # models per core
NB = 4  # model blocks
NHALF = N // 2  # 8192 points per core
CH = 1024  # chunk of batch points processed at once
NCH = NHALF // CH  # 8 chunks
MM_N = 512  # matmul moving free dim (one PSUM bank of fp32)
NEV = (MPC + 1) // 2  # 13 even-index models (head half A)
NOD = MPC // 2  # 12 odd-index models (head half B)

_CACHE: dict = {}


def _build_module():
    import concourse.bacc as bacc
    import concourse.mybir as mybir
    import concourse.tile as tile

    f32 = mybir.dt.float32
    f32m = (
        mybir.dt.float32
        if os.environ.get("KERNEL_MM_FP32", "0") == "1"
        else mybir.dt.float32r
    )
    AF = mybir.ActivationFunctionType
    ALU = mybir.AluOpType

    nc = bacc.Bacc(
        "TRN2",
        target_bir_lowering=False,
        debug=False,
        num_devices=NCORES,
    )

    NBLK = (MPC + 3) // 4  # 7 row-tiling blocks of up to 4 models
    xt_d = nc.dram_tensor("xt", [128, NHALF], f32m, kind="ExternalInput")
    w1t_d = nc.dram_tensor("w1t", [128, NBLK * H], f32m, kind="ExternalInput")
    wht_d = nc.dram_tensor("wht", [H, MPC * NH * H], f32m, kind="ExternalInput")
    whd_d = nc.dram_tensor("whd", [H, MPC * 64], f32m, kind="ExternalInput")
    b1_d = nc.dram_tensor("b1", [H, MPC], f32, kind="ExternalInput")
    bh_d = nc.dram_tensor("bh", [H, MPC * NH], f32, kind="ExternalInput")
    bhd_d = nc.dram_tensor("bhd", [64, 1], f32, kind="ExternalInput")
    mu_d = nc.dram_tensor("mu", [MPC, NHALF], f32, kind="ExternalOutput")
    sig_d = nc.dram_tensor("sig", [MPC, NHALF], f32, kind="ExternalOutput")

    with tile.TileContext(nc) as tc:
        with (
            tc.tile_pool(name="const", bufs=1) as const,
            tc.tile_pool(name="hpool", bufs=14) as hpool,
            tc.tile_pool(name="opool", bufs=2) as opool,
            tc.tile_pool(name="mmpsum", bufs=3, space="PSUM") as mmpsum,
            tc.tile_pool(name="hdpsum", bufs=1, space="PSUM") as hdpsum,
        ):
            xt = const.tile([128, NHALF], f32m)
            w1t = const.tile([128, NBLK * H], f32m)
            wht = const.tile([H, MPC * NH * H], f32m)
            whd = const.tile([H, MPC * 64], f32m)
            b1 = const.tile([H, MPC], f32)
            bh = const.tile([H, MPC * NH], f32)
            bhd = const.tile([64, 1], f32)

            nc.sync.dma_start(w1t[:], w1t_d[:])
            nc.sync.dma_start(b1[:], b1_d[:])
            nc.sync.dma_start(bh[:], bh_d[:])
            nc.sync.dma_start(bhd[:], bhd_d[:])
            nc.sync.dma_start(whd[:], whd_d[:])
            # chunked so the first models' matmuls don't wait on the full blob
            for m in range(MPC):
                s = m * NH * H
                nc.sync.dma_start(wht[:, s : s + NH * H], wht_d[:, s : s + NH * H])
            for nt in range(NCH):
                s = nt * CH
                nc.sync.dma_start(xt[:, s : s + CH], xt_d[:, s : s + CH])

            def relu_act(dst, src, bias_ap):
                nc.scalar.activation(dst, src, AF.Relu, bias=bias_ap)

            def relu_dve(dst, src, bias_ap):
                nc.vector.tensor_scalar(dst, src, bias_ap, 0.0, ALU.add, ALU.max)

            groups = [list(range(b * 4, min(b * 4 + 4, MPC))) for b in range(NBLK)]
            units = [(nt, bi) for nt in range(NCH) for bi in range(len(groups))]
            h_l1 = {}

            def emit_l1(nt, bi):
                c0 = nt * CH
                grp = groups[bi]
                for m in grp:
                    h_l1[(nt, m)] = hpool.tile([128, CH], f32m, tag="h", name="h")
                for s in range(0, CH, MM_N):
                    tiles = [
                        mmpsum.tile([128, CH], f32, tag="mm", name=f"l1ps{k}")
                        for k in range((len(grp) + 1) // 2)
                    ]
                    for j, m in enumerate(grp):
                        t = tiles[j // 2]
                        reg = t[:, (j % 2) * MM_N : (j % 2 + 1) * MM_N]
                        nc.tensor.matmul(
                            reg,
                            w1t[32 * j : 32 * j + D, bi * H : (bi + 1) * H],
                            xt[32 * j : 32 * j + D, c0 + s : c0 + s + MM_N],
                            start=True,
                            stop=True,
                            tile_position=(32 * j, 0),
                        )
                    for j, m in enumerate(grp):
                        t = tiles[j // 2]
                        reg = t[:, (j % 2) * MM_N : (j % 2 + 1) * MM_N]
                        rl = relu_act if (j + s // MM_N) % 2 == 0 else relu_dve
                        rl(h_l1[(nt, m)][:, s : s + MM_N], reg, b1[:, m : m + 1])

            emit_l1(*units[0])
            hp = None
            pending_ep = []

            def emit_epilogue(hp_t, c0_t):
                mu_t = opool.tile([MPC, CH], f32, tag="mu")
                sig_t = opool.tile([MPC, CH], f32, tag="sig")
                nc.scalar.activation(
                    mu_t[:], hp_t[0:MPC, :], AF.Identity, bias=bhd[0:MPC, :]
                )
                nc.scalar.activation(
                    sig_t[:], hp_t[32 : 32 + MPC, :], AF.Exp,
                    bias=bhd[32 : 32 + MPC, :],
                )
                nc.sync.dma_start(mu_d[:, c0_t : c0_t + CH], mu_t[:])
                nc.sync.dma_start(sig_d[:, c0_t : c0_t + CH], sig_t[:])

            for u, (nt, bi) in enumerate(units):
                c0 = nt * CH
                grp = groups[bi]
                if bi == 0:
                    hp = hdpsum.tile([64, CH], f32, tag="hp", name="hp")
                hcur = {m: h_l1.pop((nt, m)) for m in grp}
                # hidden layers, interleaved across the group
                for i in range(NH):
                    for m in grp:
                        ps = mmpsum.tile([128, CH], f32, tag="mm")
                        lhsh = wht[:, (m * NH + i) * H : (m * NH + i + 1) * H]
                        for s in range(0, CH, MM_N):
                            nc.tensor.matmul(
                                ps[:, s : s + MM_N],
                                lhsh,
                                hcur[m][:, s : s + MM_N],
                                start=True,
                                stop=True,
                            )
                        hn = hpool.tile([128, CH], f32m, tag="h")
                        bias_ap = bh[:, m * NH + i : m * NH + i + 1]
                        # alternate engines per (model, layer); every 8th model
                        # gives one extra layer to ACT to balance totals
                        on_act = (m + i) % 2 == 0 or (m % 8 == 0 and i == 1)
                        rl = relu_act if on_act else relu_dve
                        rl(hn[:], ps[:], bias_ap)
                        hcur[m] = hn
                    if i == 0 and pending_ep:
                        # previous chunk's mu/sigma finish, emitted here so the
                        # boundary ReLUs aren't queued behind them
                        emit_epilogue(*pending_ep.pop())
                    if i == NH - 2 and u + 1 < len(units):
                        # prefetch next unit's layer-1 while this unit finishes
                        emit_l1(*units[u + 1])
                # heads: accumulate all 25 models into one [64, CH] psum
                for m in grp:
                    lhshd = whd[:, m * 64 : (m + 1) * 64]
                    for s in range(0, CH, MM_N):
                        nc.tensor.matmul(
                            hp[:, s : s + MM_N],
                            lhshd,
                            hcur[m][:, s : s + MM_N],
                            start=(m == 0),
                            stop=(m == MPC - 1),
                        )
                if bi == len(groups) - 1:
                    pending_ep.append((hp, c0))
            while pending_ep:
                emit_epilogue(*pending_ep.pop())

    nc.compile()
    return nc


def _get_module():
    if "nc" not in _CACHE:
        _CACHE["nc"] = _build_module()
    return _CACHE["nc"]


def _shard_inputs(x, W1, b1, Wh, bh, Wmu, bmu, Wsig, bsig):
    """Build the per-core input maps (host-side layout prep)."""
    NBLK = (MPC + 3) // 4
    in_maps = []
    for c in range(NCORES):
        mb, half = c % NB, c // NB
        ms = slice(MPC * mb, MPC * (mb + 1))
        xh = x[NHALF * half : NHALF * (half + 1), :]  # [8192, 16]
        xtr = np.ascontiguousarray(xh.T)  # [16, 8192]
        xt_full = np.zeros((128, NHALF), dtype=np.float32)
        for rep in range(4):  # replicas at partition 0/32/64/96 for row tiling
            xt_full[32 * rep : 32 * rep + D, :] = xtr

        w1 = W1[ms]  # [25, 128, 16]
        w1t = np.zeros((128, NBLK * H), dtype=np.float32)
        for m in range(MPC):
            b, g = m // 4, m % 4
            w1t[32 * g : 32 * g + D, b * H : (b + 1) * H] = w1[m].T

        wh = Wh[ms]  # [25, 4, 128, 128] (out, in)
        wht = np.ascontiguousarray(
            wh.transpose(3, 0, 1, 2).reshape(H, MPC * NH * H)
        )  # [h_in, (m, i, h_out)]

        whd = np.zeros((H, MPC * 64), dtype=np.float32)
        for m in range(MPC):
            base = m * 64
            whd[:, base + m] = Wmu[ms][m, 0, :]
            whd[:, base + 32 + m] = Wsig[ms][m, 0, :]

        b1p = np.ascontiguousarray(b1[ms].T)  # [128, 25]
        bhp = np.ascontiguousarray(
            bh[ms].transpose(2, 0, 1).reshape(H, MPC * NH)
        )  # [128, (m, i)]
        bhdp = np.zeros((64, 1), dtype=np.float32)
        bhdp[0:MPC, 0] = bmu[ms][:, 0]
        bhdp[32 : 32 + MPC, 0] = bsig[ms][:, 0]

        in_maps.append(
            {
                "xt": xt_full,
                "w1t": w1t,
                "wht": wht,
                "whd": whd,
                "b1": b1p,
                "bh": bhp,
                "bhd": bhdp,
            }
        )
    return in_maps


def _run(in_maps, trace=False):
    from concourse.bass_utils import run_bass_kernel_spmd

    nc = _get_module()
    return run_bass_kernel_spmd(
        nc, in_maps, list(range(NCORES)), trace=trace
    )


def kernel(x, W1, b1, Wh, bh, Wmu, bmu, Wsig, bsig):
    args = [
        np.ascontiguousarray(np.asarray(a, dtype=np.float32))
        for a in (x, W1, b1, Wh, bh, Wmu, bmu, Wsig, bsig)
    ]
    in_maps = _shard_inputs(*args)
    res = _run(in_maps, trace=bool(int(os.environ.get("KERNEL_TRACE", "0"))))
    _CACHE["last_results"] = res

    mu = np.empty((M, N), dtype=np.float32)
    sig = np.empty((M, N), dtype=np.float32)
    for c in range(NCORES):
        mb, half = c % NB, c // NB
        m0 = MPC * mb
        ns = slice(NHALF * half, NHALF * (half + 1))
        r = res.results[c]
        mu[m0 : m0 + MPC, ns] = r["mu"]
        sig[m0 : m0 + MPC, ns] = r["sig"]
    return (mu.reshape(M, N, O), sig.reshape(M, N, O))



# revision 4
# speedup vs baseline: 1.1494x; 1.1494x over previous
"""Bootstrap-ensemble MLP (100 models, D=16 -> H=128 x5 -> mu/sigma heads)
on 8 Trainium2 NeuronCores.

Sharding: every core runs an identical SPMD program over 25 models x 8192
batch points (model axis split 4 ways x batch split 2 ways).  All per-core
weights are pre-arranged on the host into the exact SBUF layouts the
TensorEngine wants (lhsT = pre-transposed stationary operand).

Key perf structure:
- fp16 matmuls (1 cycle/column on the PE vs ~2.2 for fp32r), fp32 PSUM
  accumulate + fp32 bias -> accuracy stays ~1e-3.
- layer-1 bias folded into the matmul via a ones-row (K=17): the L1
  ReLU needs no per-model bias operand, so one fused op covers the two
  models sharing a PSUM tile.
- bias+ReLU ops are greedily balanced across ACT (1.2 GHz) and DVE
  (0.96 GHz) using measured per-op costs (only those two engines can
  read PSUM).
- mu/sigma head matmuls accumulate all 25 models into one [64, CH] PSUM
  tile via zero-padded per-model head weights; finished with
  Identity/Exp activations with the bias folded in.
"""

import os

import numpy as np

M = 100  # n_models
D = 16  # input_dim
H = 128  # hidden_dim
O = 1  # output_dim
NH = 4  # n_hidden
N = 16384  # batch of query points

NCORES = 8
MPC = 25  # models per core
NB = 4  # model blocks
NHALF = N // 2  # 8192 points per core
CH = 1024  # chunk of batch points processed at once
NCH = NHALF // CH  # 8 chunks
MM_N = 512  # matmul moving free dim (one PSUM bank of fp32)
KL1 = D + 1  # L1 contraction rows: 16 inputs + 1 ones-row for the bias

_CACHE: dict = {}


def _build_module():
    import concourse.bacc as bacc
    import concourse.mybir as mybir
    import concourse.tile as tile

    f32 = mybir.dt.float32
    f16 = mybir.dt.float16
    AF = mybir.ActivationFunctionType
    ALU = mybir.AluOpType

    nc = bacc.Bacc(
        "TRN2",
        target_bir_lowering=False,
        debug=False,
        num_devices=NCORES,
    )

    NBLK = (MPC + 3) // 4  # 7 row-tiling blocks of up to 4 models
    xt_d = nc.dram_tensor("xt", [128, NHALF], f16, kind="ExternalInput")
    w1t_d = nc.dram_tensor("w1t", [128, NBLK * H], f16, kind="ExternalInput")
    wht_d = nc.dram_tensor("wht", [H, MPC * NH * H], f16, kind="ExternalInput")
    whd_d = nc.dram_tensor("whd", [H, MPC * 64], f16, kind="ExternalInput")
    bh_d = nc.dram_tensor("bh", [H, MPC * NH], f32, kind="ExternalInput")
    bhd_d = nc.dram_tensor("bhd", [64, 1], f32, kind="ExternalInput")
    mu_d = nc.dram_tensor("mu", [MPC, NHALF], f32, kind="ExternalOutput")
    sig_d = nc.dram_tensor("sig", [MPC, NHALF], f32, kind="ExternalOutput")

    with tile.TileContext(nc) as tc:
        with (
            tc.tile_pool(name="const", bufs=1) as const,
            tc.tile_pool(name="ppool", bufs=6) as ppool,
            tc.tile_pool(name="hpool", bufs=14) as hpool,
            tc.tile_pool(name="opool", bufs=2) as opool,
            tc.tile_pool(name="mmpsum", bufs=3, space="PSUM") as mmpsum,
            tc.tile_pool(name="hdpsum", bufs=1, space="PSUM") as hdpsum,
        ):
            xt = const.tile([128, NHALF], f16)
            w1t = const.tile([128, NBLK * H], f16)
            wht = const.tile([H, MPC * NH * H], f16)
            whd = const.tile([H, MPC * 64], f16)
            bh = const.tile([H, MPC * NH], f32)
            bhd = const.tile([64, 1], f32)

            nc.sync.dma_start(w1t[:], w1t_d[:])
            nc.sync.dma_start(bh[:], bh_d[:])
            nc.sync.dma_start(bhd[:], bhd_d[:])
            nc.sync.dma_start(whd[:], whd_d[:])
            # chunked so the first models' matmuls don't wait on the full blob
            for m in range(MPC):
                s = m * NH * H
                nc.sync.dma_start(wht[:, s : s + NH * H], wht_d[:, s : s + NH * H])
            for nt in range(NCH):
                s = nt * CH
                nc.sync.dma_start(xt[:, s : s + CH], xt_d[:, s : s + CH])

            # --- greedy ACT/DVE balance (measured per-op cost, ns) ---
            eng_t = {"act": 0.0, "dve": 0.0}

            def cost(eng, cols):
                return cols * 0.836 + 257.0 if eng == "act" else cols * 1.035 + 215.0

            def relu(dst, src, bias_ap=None, cols=CH, pin=None):
                e = pin or min(eng_t, key=lambda k: eng_t[k] + cost(k, cols))
                eng_t[e] += cost(e, cols)
                if e == "act":
                    nc.scalar.activation(
                        dst, src, AF.Relu,
                        bias=bias_ap if bias_ap is not None else 0.0,
                    )
                else:
                    nc.vector.tensor_scalar(
                        dst, src, bias_ap if bias_ap is not None else 0.0,
                        0.0, ALU.add, ALU.max,
                    )

            groups = [list(range(b * 4, min(b * 4 + 4, MPC))) for b in range(NBLK)]
            units = [(nt, bi) for nt in range(NCH) for bi in range(len(groups))]
            h_l1 = {}

            def emit_l1(nt, bi):
                c0 = nt * CH
                grp = groups[bi]
                npair = (len(grp) + 1) // 2
                pts = [ppool.tile([128, 2, CH], f16, tag="hp2", name="hpair") for _ in range(npair)]
                for j, m in enumerate(grp):
                    h_l1[(nt, m)] = (pts[j // 2], j % 2)
                for s in range(0, CH, MM_N):
                    tiles = [
                        mmpsum.tile([128, 2, MM_N], f32, tag="mm", name=f"l1ps{k}")
                        for k in range(npair)
                    ]
                    for j, m in enumerate(grp):
                        nc.tensor.matmul(
                            tiles[j // 2][:, j % 2, :],
                            w1t[32 * j : 32 * j + KL1, bi * H : (bi + 1) * H],
                            xt[32 * j : 32 * j + KL1, c0 + s : c0 + s + MM_N],
                            start=True,
                            stop=True,
                            tile_position=(32 * j, 0),
                        )
                    for k in range(npair):
                        nm = min(2, len(grp) - 2 * k)
                        relu(
                            pts[k][:, 0:nm, s : s + MM_N],
                            tiles[k][:, 0:nm, :],
                            cols=nm * MM_N,
                        )

            emit_l1(*units[0])
            hp = None
            pending_ep = []

            def emit_epilogue(hp_t, c0_t):
                mu_t = opool.tile([MPC, CH], f32, tag="mu")
                sig_t = opool.tile([MPC, CH], f32, tag="sig")
                eng_t["dve"] += cost("dve", CH)
                nc.vector.tensor_scalar(
                    mu_t[:], hp_t[0:MPC, :], bhd[0:MPC, :], 0.0, ALU.add, ALU.bypass
                )
                eng_t["act"] += cost("act", CH)
                nc.scalar.activation(
                    sig_t[:], hp_t[32 : 32 + MPC, :], AF.Exp,
                    bias=bhd[32 : 32 + MPC, :],
                )
                nc.sync.dma_start(mu_d[:, c0_t : c0_t + CH], mu_t[:])
                nc.sync.dma_start(sig_d[:, c0_t : c0_t + CH], sig_t[:])

            for u, (nt, bi) in enumerate(units):
                c0 = nt * CH
                grp = groups[bi]
                if bi == 0:
                    hp = hdpsum.tile([64, CH], f32, tag="hp", name="hp")
                hcur = {m: h_l1.pop((nt, m)) for m in grp}

                def rhs(m, s):
                    t = hcur[m]
                    if isinstance(t, tuple):
                        return t[0][:, t[1], s : s + MM_N]
                    return t[:, s : s + MM_N]

                # hidden layers, interleaved across the group
                for i in range(NH):
                    for m in grp:
                        ps = mmpsum.tile([128, CH], f32, tag="mm")
                        lhsh = wht[:, (m * NH + i) * H : (m * NH + i + 1) * H]
                        for s in range(0, CH, MM_N):
                            nc.tensor.matmul(
                                ps[:, s : s + MM_N],
                                lhsh,
                                rhs(m, s),
                                start=True,
                                stop=True,
                            )
                        hn = hpool.tile([128, CH], f16, tag="h")
                        relu(hn[:], ps[:], bias_ap=bh[:, m * NH + i : m * NH + i + 1])
                        hcur[m] = hn
                    if i == 0 and pending_ep:
                        # previous chunk's mu/sigma finish, emitted here so the
                        # boundary ReLUs aren't queued behind them
                        emit_epilogue(*pending_ep.pop())
                    if i == NH - 2 and u + 1 < len(units):
                        # prefetch next unit's layer-1 while this unit finishes
                        emit_l1(*units[u + 1])
                # heads: accumulate all 25 models into one [64, CH] psum
                for m in grp:
                    lhshd = whd[:, m * 64 : (m + 1) * 64]
                    for s in range(0, CH, MM_N):
                        nc.tensor.matmul(
                            hp[:, s : s + MM_N],
                            lhshd,
                            rhs(m, s),
                            start=(m == 0),
                            stop=(m == MPC - 1),
                        )
                if bi == len(groups) - 1:
                    pending_ep.append((hp, c0))
            while pending_ep:
                emit_epilogue(*pending_ep.pop())

    nc.compile()
    return nc


def _get_module():
    if "nc" not in _CACHE:
        _CACHE["nc"] = _build_module()
    return _CACHE["nc"]


def _shard_inputs(x, W1, b1, Wh, bh, Wmu, bmu, Wsig, bsig):
    """Build the per-core input maps (host-side layout prep)."""
    NBLK = (MPC + 3) // 4
    in_maps = []
    for c in range(NCORES):
        mb, half = c % NB, c // NB
        ms = slice(MPC * mb, MPC * (mb + 1))
        xh = x[NHALF * half : NHALF * (half + 1), :]  # [8192, 16]
        xtr = np.ascontiguousarray(xh.T)  # [16, 8192]
        xt_full = np.zeros((128, NHALF), dtype=np.float16)
        for rep in range(4):  # replicas at partition 0/32/64/96 for row tiling
            xt_full[32 * rep : 32 * rep + D, :] = xtr
            xt_full[32 * rep + D, :] = 1.0  # ones-row: bias via matmul

        w1 = W1[ms]  # [25, 128, 16]
        b1c = b1[ms]  # [25, 128]
        w1t = np.zeros((128, NBLK * H), dtype=np.float16)
        for m in range(MPC):
            b, g = m // 4, m % 4
            w1t[32 * g : 32 * g + D, b * H : (b + 1) * H] = w1[m].T
            w1t[32 * g + D, b * H : (b + 1) * H] = b1c[m]

        wh = Wh[ms]  # [25, 4, 128, 128] (out, in)
        wht = np.ascontiguousarray(
            wh.transpose(3, 0, 1, 2).reshape(H, MPC * NH * H)
        ).astype(np.float16)  # [h_in, (m, i, h_out)]

        whd = np.zeros((H, MPC * 64), dtype=np.float16)
        for m in range(MPC):
            base = m * 64
            whd[:, base + m] = Wmu[ms][m, 0, :]
            whd[:, base + 32 + m] = Wsig[ms][m, 0, :]

        bhp = np.ascontiguousarray(
            bh[ms].transpose(2, 0, 1).reshape(H, MPC * NH)
        )  # [128, (m, i)]
        bhdp = np.zeros((64, 1), dtype=np.float32)
        bhdp[0:MPC, 0] = bmu[ms][:, 0]
        bhdp[32 : 32 + MPC, 0] = bsig[ms][:, 0]

        in_maps.append(
            {
                "xt": xt_full,
                "w1t": w1t,
                "wht": wht,
                "whd": whd,
                "bh": bhp,
                "bhd": bhdp,
            }
        )
    return in_maps


def _run(in_maps, trace=False):
    from concourse.bass_utils import run_bass_kernel_spmd

    nc = _get_module()
    return run_bass_kernel_spmd(
        nc, in_maps, list(range(NCORES)), trace=trace
    )


def kernel(x, W1, b1, Wh, bh, Wmu, bmu, Wsig, bsig):
    args = [
        np.ascontiguousarray(np.asarray(a, dtype=np.float32))
        for a in (x, W1, b1, Wh, bh, Wmu, bmu, Wsig, bsig)
    ]
    in_maps = _shard_inputs(*args)
    res = _run(in_maps, trace=bool(int(os.environ.get("KERNEL_TRACE", "0"))))
    _CACHE["last_results"] = res

    mu = np.empty((M, N), dtype=np.float32)
    sig = np.empty((M, N), dtype=np.float32)
    for c in range(NCORES):
        mb, half = c % NB, c // NB
        m0 = MPC * mb
        ns = slice(NHALF * half, NHALF * (half + 1))
        r = res.results[c]
        mu[m0 : m0 + MPC, ns] = r["mu"]
        sig[m0 : m0 + MPC, ns] = r["sig"]
    return (mu.reshape(M, N, O), sig.reshape(M, N, O))


# revision 8
# speedup vs baseline: 1.1776x; 1.0245x over previous
"""Bootstrap-ensemble MLP (100 models, D=16 -> H=128 x5 -> mu/sigma heads)
on 8 Trainium2 NeuronCores.

Sharding: every core runs an identical SPMD program over 25 models x 8192
batch points (model axis split 4 ways x batch split 2 ways).  All per-core
weights are pre-arranged on the host into the exact SBUF layouts the
TensorEngine wants (lhsT = pre-transposed stationary operand).

Key perf structure:
- fp16 matmuls (1 cycle/column on the PE vs ~2.2 for fp32r), fp32 PSUM
  accumulate + fp32 bias -> accuracy stays ~1e-3.
- layer-1 bias folded into the matmul via a ones-row (K=17): the L1
  ReLU needs no per-model bias operand, so one fused op covers the two
  models sharing a PSUM tile.
- bias+ReLU ops are greedily balanced across ACT (1.2 GHz) and DVE
  (0.96 GHz) using measured per-op costs (only those two engines can
  read PSUM).
- mu/sigma head matmuls accumulate all 25 models into one [64, CH] PSUM
  tile via zero-padded per-model head weights; finished with
  Identity/Exp activations with the bias folded in.
"""

import os

import numpy as np

M = 100  # n_models
D = 16  # input_dim
H = 128  # hidden_dim
O = 1  # output_dim
NH = 4  # n_hidden
N = 16384  # batch of query points

NCORES = 8
MPC = 25  # models per core
NB = 4  # model blocks
NHALF = N // 2  # 8192 points per core
CH = 1024  # chunk of batch points processed at once
NCH = NHALF // CH  # 8 chunks
MM_N = 512  # matmul moving free dim (one PSUM bank of fp32)
KL1 = D + 1  # L1 contraction rows: 16 inputs + 1 ones-row for the bias

_CACHE: dict = {}


def _build_module():
    import concourse.bacc as bacc
    import concourse.mybir as mybir
    import concourse.tile as tile

    f32 = mybir.dt.float32
    f16 = mybir.dt.float16
    AF = mybir.ActivationFunctionType
    ALU = mybir.AluOpType

    nc = bacc.Bacc(
        "TRN2",
        target_bir_lowering=False,
        debug=False,
        num_devices=NCORES,
    )

    NBLK = (MPC + 3) // 4  # 7 row-tiling blocks of up to 4 models
    xt_d = nc.dram_tensor("xt", [128, NHALF], f16, kind="ExternalInput")
    w1t_d = nc.dram_tensor("w1t", [128, NBLK * H], f16, kind="ExternalInput")
    wht_d = nc.dram_tensor("wht", [H, MPC * NH * H], f16, kind="ExternalInput")
    whd_d = nc.dram_tensor("whd", [H, MPC * 64], f16, kind="ExternalInput")
    bh_d = nc.dram_tensor("bh", [H, MPC * NH], f32, kind="ExternalInput")
    bhd_d = nc.dram_tensor("bhd", [64, 1], f32, kind="ExternalInput")
    mu_d = nc.dram_tensor("mu", [MPC, NHALF], f32, kind="ExternalOutput")
    sig_d = nc.dram_tensor("sig", [MPC, NHALF], f32, kind="ExternalOutput")

    with tile.TileContext(nc) as tc:
        with (
            tc.tile_pool(name="const", bufs=1) as const,
            tc.tile_pool(name="ppool", bufs=6) as ppool,
            tc.tile_pool(name="hpool", bufs=14) as hpool,
            tc.tile_pool(name="opool", bufs=2) as opool,
            tc.tile_pool(name="mmpsum", bufs=3, space="PSUM") as mmpsum,
            tc.tile_pool(name="hdpsum", bufs=1, space="PSUM") as hdpsum,
        ):
            xt = const.tile([128, NHALF], f16)
            w1t = const.tile([128, NBLK * H], f16)
            wht = const.tile([H, MPC * NH * H], f16)
            whd = const.tile([H, MPC * 64], f16)
            bh = const.tile([H, MPC * NH], f32)
            bhd = const.tile([64, 1], f32)

            nc.sync.dma_start(w1t[:], w1t_d[:])
            nc.sync.dma_start(bh[:], bh_d[:])
            nc.sync.dma_start(bhd[:], bhd_d[:])
            nc.sync.dma_start(whd[:], whd_d[:])
            # chunked so the first models' matmuls don't wait on the full blob
            for m in range(MPC):
                s = m * NH * H
                nc.sync.dma_start(wht[:, s : s + NH * H], wht_d[:, s : s + NH * H])
            for nt in range(NCH):
                s = nt * CH
                nc.sync.dma_start(xt[:, s : s + CH], xt_d[:, s : s + CH])

            # --- greedy ACT/DVE balance (measured per-op cost, ns) ---
            eng_t = {"act": 0.0, "dve": 0.0}

            def cost(eng, cols):
                return cols * 0.836 + 257.0 if eng == "act" else cols * 1.035 + 215.0

            def relu(dst, src, bias_ap=None, cols=CH, pin=None):
                e = pin or min(eng_t, key=lambda k: eng_t[k] + cost(k, cols))
                eng_t[e] += cost(e, cols)
                if e == "act":
                    nc.scalar.activation(
                        dst, src, AF.Relu,
                        bias=bias_ap if bias_ap is not None else 0.0,
                    )
                else:
                    nc.vector.tensor_scalar(
                        dst, src, bias_ap if bias_ap is not None else 0.0,
                        0.0, ALU.add, ALU.max,
                    )

            groups = [list(range(b * 4, min(b * 4 + 4, MPC))) for b in range(NBLK)]
            units = [(nt, bi) for nt in range(NCH) for bi in range(len(groups))]
            h_l1 = {}

            # side-work queue: L1-prefetch and head matmuls are emitted one
            # item at a time between hidden-layer slots, so the PE's hidden
            # PSUM-tile stream (which feeds ACT/DVE) never pauses for a burst
            from collections import deque

            side = deque()  # items: (tag, closure)

            def pop_side(n=1):
                for _ in range(n):
                    if side:
                        side.popleft()[1]()

            def flush_side(pred):
                keep = deque()
                while side:
                    tag, fn = side.popleft()
                    if pred(tag):
                        fn()
                    else:
                        keep.append((tag, fn))
                side.extend(keep)

            def emit_l1_pair(nt, bi, k, s):
                c0 = nt * CH
                grp = groups[bi]
                nm = min(2, len(grp) - 2 * k)
                if s == 0 and (nt, grp[2 * k]) not in h_l1:
                    pt = ppool.tile([128, 2, CH], f16, tag="hp2", name="hpair")
                    for jj in range(nm):
                        h_l1[(nt, grp[2 * k + jj])] = (pt, jj)
                pt = h_l1[(nt, grp[2 * k])][0]
                ps = mmpsum.tile([128, 2, MM_N], f32, tag="mm", name="l1ps")
                for jj in range(nm):
                    j = 2 * k + jj
                    nc.tensor.matmul(
                        ps[:, jj, :],
                        w1t[32 * j : 32 * j + KL1, bi * H : (bi + 1) * H],
                        xt[32 * j : 32 * j + KL1, c0 + s : c0 + s + MM_N],
                        start=True,
                        stop=True,
                        tile_position=(32 * j, 0),
                    )
                relu(pt[:, 0:nm, s : s + MM_N], ps[:, 0:nm, :], cols=nm * MM_N)

            def enqueue_l1(nt, bi):
                grp = groups[bi]
                for s in range(0, CH, MM_N):
                    for k in range((len(grp) + 1) // 2):
                        side.append(
                            (("l1", nt, bi), lambda k=k, s=s: emit_l1_pair(nt, bi, k, s))
                        )

            def emit_l1_now(nt, bi):
                flush_side(lambda t: t == ("l1", nt, bi))

            enqueue_l1(*units[0])
            emit_l1_now(*units[0])
            hp = None
            pending_ep = []

            def emit_epilogue(hp_t, c0_t):
                mu_t = opool.tile([MPC, CH], f32, tag="mu")
                sig_t = opool.tile([MPC, CH], f32, tag="sig")
                eng_t["dve"] += cost("dve", CH)
                nc.vector.tensor_scalar(
                    mu_t[:], hp_t[0:MPC, :], bhd[0:MPC, :], 0.0, ALU.add, ALU.bypass
                )
                eng_t["act"] += cost("act", CH)
                nc.scalar.activation(
                    sig_t[:], hp_t[32 : 32 + MPC, :], AF.Exp,
                    bias=bhd[32 : 32 + MPC, :],
                )
                nc.sync.dma_start(mu_d[:, c0_t : c0_t + CH], mu_t[:])
                nc.sync.dma_start(sig_d[:, c0_t : c0_t + CH], sig_t[:])

            for u, (nt, bi) in enumerate(units):
                c0 = nt * CH
                grp = groups[bi]
                if bi == 0:
                    hp = hdpsum.tile([64, CH], f32, tag="hp", name="hp")
                # make sure this unit's layer-1 has been emitted (normally a
                # no-op: the items drained via pop_side during the prior unit)
                emit_l1_now(nt, bi)
                hcur = {m: h_l1.pop((nt, m)) for m in grp}

                def rhs(m, s):
                    t = hcur[m]
                    if isinstance(t, tuple):
                        return t[0][:, t[1], s : s + MM_N]
                    return t[:, s : s + MM_N]

                def enqueue_heads(m, hp=None, hcur=None):
                    def emit(
                        hp=hp,
                        t=hcur[m],
                        m=m,
                        st=(m == grp[0] and bi == 0),
                        sp=(m == grp[-1] and bi == len(groups) - 1),
                    ):
                        for s in range(0, CH, MM_N):
                            nc.tensor.matmul(
                                hp[:, s : s + MM_N],
                                whd[:, m * 64 : (m + 1) * 64],
                                t[:, s : s + MM_N],
                                start=st,
                                stop=sp,
                            )

                    side.append((("hd", nt), emit))

                # hidden layers, interleaved across the group
                for i in range(NH):
                    for m in grp:
                        ps = mmpsum.tile([128, CH], f32, tag="mm")
                        lhsh = wht[:, (m * NH + i) * H : (m * NH + i + 1) * H]
                        for s in range(0, CH, MM_N):
                            nc.tensor.matmul(
                                ps[:, s : s + MM_N],
                                lhsh,
                                rhs(m, s),
                                start=True,
                                stop=True,
                            )
                        hn = hpool.tile([128, CH], f16, tag="h")
                        relu(hn[:], ps[:], bias_ap=bh[:, m * NH + i : m * NH + i + 1])
                        hcur[m] = hn
                        pop_side()
                        if i == NH - 1:
                            enqueue_heads(m, hp=hp, hcur=hcur)
                    if i == 0 and pending_ep:
                        # previous chunk's mu/sigma finish; its head matmuls
                        # must be fully emitted first (in-order ACT queue)
                        hp_t, c0_t, nt_t = pending_ep.pop()
                        flush_side(lambda t: t == ("hd", nt_t))
                        emit_epilogue(hp_t, c0_t)
                    if i == 1 and u + 1 < len(units):
                        # prefetch next unit's layer-1, spread via the queue
                        enqueue_l1(*units[u + 1])
                if bi == len(groups) - 1:
                    pending_ep.append((hp, c0, nt))
            while pending_ep:
                hp_t, c0_t, nt_t = pending_ep.pop()
                flush_side(lambda t: t == ("hd", nt_t))
                emit_epilogue(hp_t, c0_t)

    nc.compile()
    return nc


def _get_module():
    if "nc" not in _CACHE:
        _CACHE["nc"] = _build_module()
    return _CACHE["nc"]


def _shard_inputs(x, W1, b1, Wh, bh, Wmu, bmu, Wsig, bsig):
    """Build the per-core input maps (host-side layout prep)."""
    NBLK = (MPC + 3) // 4
    in_maps = []
    for c in range(NCORES):
        mb, half = c % NB, c // NB
        ms = slice(MPC * mb, MPC * (mb + 1))
        xh = x[NHALF * half : NHALF * (half + 1), :]  # [8192, 16]
        xtr = np.ascontiguousarray(xh.T)  # [16, 8192]
        xt_full = np.zeros((128, NHALF), dtype=np.float16)
        for rep in range(4):  # replicas at partition 0/32/64/96 for row tiling
            xt_full[32 * rep : 32 * rep + D, :] = xtr
            xt_full[32 * rep + D, :] = 1.0  # ones-row: bias via matmul

        w1 = W1[ms]  # [25, 128, 16]
        b1c = b1[ms]  # [25, 128]
        w1t = np.zeros((128, NBLK * H), dtype=np.float16)
        for m in range(MPC):
            b, g = m // 4, m % 4
            w1t[32 * g : 32 * g + D, b * H : (b + 1) * H] = w1[m].T
            w1t[32 * g + D, b * H : (b + 1) * H] = b1c[m]

        wh = Wh[ms]  # [25, 4, 128, 128] (out, in)
        wht = np.ascontiguousarray(
            wh.transpose(3, 0, 1, 2).reshape(H, MPC * NH * H)
        ).astype(np.float16)  # [h_in, (m, i, h_out)]

        whd = np.zeros((H, MPC * 64), dtype=np.float16)
        for m in range(MPC):
            base = m * 64
            whd[:, base + m] = Wmu[ms][m, 0, :]
            whd[:, base + 32 + m] = Wsig[ms][m, 0, :]

        bhp = np.ascontiguousarray(
            bh[ms].transpose(2, 0, 1).reshape(H, MPC * NH)
        )  # [128, (m, i)]
        bhdp = np.zeros((64, 1), dtype=np.float32)
        bhdp[0:MPC, 0] = bmu[ms][:, 0]
        bhdp[32 : 32 + MPC, 0] = bsig[ms][:, 0]

        in_maps.append(
            {
                "xt": xt_full,
                "w1t": w1t,
                "wht": wht,
                "whd": whd,
                "bh": bhp,
                "bhd": bhdp,
            }
        )
    return in_maps


def _run(in_maps, trace=False):
    from concourse.bass_utils import run_bass_kernel_spmd

    nc = _get_module()
    return run_bass_kernel_spmd(
        nc, in_maps, list(range(NCORES)), trace=trace
    )


def kernel(x, W1, b1, Wh, bh, Wmu, bmu, Wsig, bsig):
    args = [
        np.ascontiguousarray(np.asarray(a, dtype=np.float32))
        for a in (x, W1, b1, Wh, bh, Wmu, bmu, Wsig, bsig)
    ]
    in_maps = _shard_inputs(*args)
    res = _run(in_maps, trace=bool(int(os.environ.get("KERNEL_TRACE", "0"))))
    _CACHE["last_results"] = res

    mu = np.empty((M, N), dtype=np.float32)
    sig = np.empty((M, N), dtype=np.float32)
    for c in range(NCORES):
        mb, half = c % NB, c // NB
        m0 = MPC * mb
        ns = slice(NHALF * half, NHALF * (half + 1))
        r = res.results[c]
        mu[m0 : m0 + MPC, ns] = r["mu"]
        sig[m0 : m0 + MPC, ns] = r["sig"]
    return (mu.reshape(M, N, O), sig.reshape(M, N, O))


# revision 12
# speedup vs baseline: 1.2004x; 1.0193x over previous
"""Bootstrap-ensemble MLP (100 models, D=16 -> H=128 x5 -> mu/sigma heads)
on 8 Trainium2 NeuronCores.

Sharding: every core runs an identical SPMD program over 25 models x 8192
batch points (model axis split 4 ways x batch split 2 ways).  All per-core
weights are pre-arranged on the host into the exact SBUF layouts the
TensorEngine wants (lhsT = pre-transposed stationary operand).

Key perf structure:
- fp16 matmuls (1 cycle/column on the PE vs ~2.2 for fp32r), fp32 PSUM
  accumulate + fp32 bias -> accuracy stays ~1e-3.
- layer-1 bias folded into the matmul via a ones-row (K=17): the L1
  ReLU needs no per-model bias operand, so one fused op covers the two
  models sharing a PSUM tile.
- bias+ReLU ops are greedily balanced across ACT (1.2 GHz) and DVE
  (0.96 GHz) using measured per-op costs (only those two engines can
  read PSUM).
- mu/sigma head matmuls accumulate all 25 models into one [64, CH] PSUM
  tile via zero-padded per-model head weights; finished with
  Identity/Exp activations with the bias folded in.
"""

import os

import numpy as np

M = 100  # n_models
D = 16  # input_dim
H = 128  # hidden_dim
O = 1  # output_dim
NH = 4  # n_hidden
N = 16384  # batch of query points

NCORES = 8
MPC = 25  # models per core
NB = 4  # model blocks
NHALF = N // 2  # 8192 points per core
CH = 1024  # chunk of batch points processed at once
NCH = NHALF // CH  # 8 chunks
MM_N = 512  # matmul moving free dim (one PSUM bank of fp32)
KL1 = D + 1  # L1 contraction rows: 16 inputs + 1 ones-row for the bias

_CACHE: dict = {}


def _build_module():
    import concourse.bacc as bacc
    import concourse.mybir as mybir
    import concourse.tile as tile

    f32 = mybir.dt.float32
    f16 = mybir.dt.float16
    AF = mybir.ActivationFunctionType
    ALU = mybir.AluOpType

    nc = bacc.Bacc(
        "TRN2",
        target_bir_lowering=False,
        debug=False,
        num_devices=NCORES,
    )

    NBLK = (MPC + 3) // 4  # 7 row-tiling blocks of up to 4 models
    xt_d = nc.dram_tensor("xt", [128, NHALF], f16, kind="ExternalInput")
    w1t_d = nc.dram_tensor("w1t", [128, NBLK * H], f16, kind="ExternalInput")
    wht_d = nc.dram_tensor("wht", [H, MPC * NH * H], f16, kind="ExternalInput")
    whd_d = nc.dram_tensor("whd", [H, MPC * 64], f16, kind="ExternalInput")
    bh_d = nc.dram_tensor("bh", [H, MPC * NH], f32, kind="ExternalInput")
    bhd_d = nc.dram_tensor("bhd", [64, 1], f32, kind="ExternalInput")
    mu_d = nc.dram_tensor("mu", [MPC, NHALF], f32, kind="ExternalOutput")
    sig_d = nc.dram_tensor("sig", [MPC, NHALF], f32, kind="ExternalOutput")

    with tile.TileContext(nc) as tc:
        with (
            tc.tile_pool(name="const", bufs=1) as const,
            tc.tile_pool(name="ppool", bufs=8) as ppool,
            tc.tile_pool(name="hpool", bufs=16) as hpool,
            tc.tile_pool(name="opool", bufs=4) as opool,
            tc.tile_pool(name="accpool", bufs=3) as accpool,
            tc.tile_pool(name="mmpsum", bufs=4, space="PSUM") as mmpsum,
        ):
            xt = const.tile([128, NHALF], f16)
            w1t = const.tile([128, NBLK * H], f16)
            wht = const.tile([H, MPC * NH * H], f16)
            whd = const.tile([H, MPC * 64], f16)
            bh = const.tile([H, MPC * NH], f32)
            bhd = const.tile([64, 1], f32)

            nc.sync.dma_start(w1t[:], w1t_d[:])
            nc.sync.dma_start(bh[:], bh_d[:])
            nc.sync.dma_start(bhd[:], bhd_d[:])
            nc.sync.dma_start(whd[:], whd_d[:])
            # chunked so the first models' matmuls don't wait on the full blob
            for m in range(MPC):
                s = m * NH * H
                nc.sync.dma_start(wht[:, s : s + NH * H], wht_d[:, s : s + NH * H])
            for nt in range(NCH):
                s = nt * CH
                nc.sync.dma_start(xt[:, s : s + CH], xt_d[:, s : s + CH])

            # --- greedy ACT/DVE balance (measured per-op cost, ns) ---
            eng_t = {"act": 0.0, "dve": 0.0}

            def cost(eng, cols):
                return cols * 0.836 + 257.0 if eng == "act" else cols * 1.035 + 215.0

            def relu(dst, src, bias_ap=None, cols=CH, pin=None):
                e = pin or min(eng_t, key=lambda k: eng_t[k] + cost(k, cols))
                eng_t[e] += cost(e, cols)
                if e == "act":
                    nc.scalar.activation(
                        dst, src, AF.Relu,
                        bias=bias_ap if bias_ap is not None else 0.0,
                    )
                else:
                    nc.vector.tensor_scalar(
                        dst, src, bias_ap if bias_ap is not None else 0.0,
                        0.0, ALU.add, ALU.max,
                    )

            groups = [list(range(b * 4, min(b * 4 + 4, MPC))) for b in range(NBLK)]
            units = [(nt, bi) for nt in range(NCH) for bi in range(len(groups))]
            h_l1 = {}

            # side-work queue: L1-prefetch and head matmuls are emitted one
            # item at a time between hidden-layer slots, so the PE's hidden
            # PSUM-tile stream (which feeds ACT/DVE) never pauses for a burst
            from collections import deque

            side = deque()  # items: (tag, closure)

            def pop_side(n=1):
                for _ in range(n):
                    if side:
                        side.popleft()[1]()

            def flush_side(pred):
                keep = deque()
                while side:
                    tag, fn = side.popleft()
                    if pred(tag):
                        fn()
                    else:
                        keep.append((tag, fn))
                side.extend(keep)

            def emit_l1_pair(nt, bi, k, s):
                c0 = nt * CH
                grp = groups[bi]
                nm = min(2, len(grp) - 2 * k)
                if s == 0 and (nt, grp[2 * k]) not in h_l1:
                    pt = ppool.tile([128, 2, CH], f16, tag="hp2", name="hpair")
                    for jj in range(nm):
                        h_l1[(nt, grp[2 * k + jj])] = (pt, jj)
                pt = h_l1[(nt, grp[2 * k])][0]
                ps = mmpsum.tile([128, 2, MM_N], f32, tag="mm", name="l1ps")
                for jj in range(nm):
                    j = 2 * k + jj
                    nc.tensor.matmul(
                        ps[:, jj, :],
                        w1t[32 * j : 32 * j + KL1, bi * H : (bi + 1) * H],
                        xt[32 * j : 32 * j + KL1, c0 + s : c0 + s + MM_N],
                        start=True,
                        stop=True,
                        tile_position=(32 * j, 0),
                    )
                relu(pt[:, 0:nm, s : s + MM_N], ps[:, 0:nm, :], cols=nm * MM_N)

            def enqueue_l1(nt, bi):
                grp = groups[bi]
                for s in range(0, CH, MM_N):
                    for k in range((len(grp) + 1) // 2):
                        side.append(
                            (("l1", nt, bi), lambda k=k, s=s: emit_l1_pair(nt, bi, k, s))
                        )

            def emit_l1_now(nt, bi):
                flush_side(lambda t: t == ("l1", nt, bi))

            enqueue_l1(*units[0])
            emit_l1_now(*units[0])
            pending_ep = []
            grp_hp = {}  # (nt, bi) -> per-group head psum partial
            chunk_acc = {}  # nt -> running SBUF head accumulator

            def emit_epilogue(acc_t, c0_t):
                mu_t = opool.tile([MPC, CH], f32, tag="mu")
                sig_t = opool.tile([MPC, CH], f32, tag="sig")
                # mu = acc + bias on the otherwise-idle GPSIMD (SBUF->SBUF)
                nc.gpsimd.tensor_scalar_add(mu_t[:], acc_t[0:MPC, :], bhd[0:MPC, :])
                eng_t["act"] += cost("act", CH)
                nc.scalar.activation(
                    sig_t[:], acc_t[32 : 32 + MPC, :], AF.Exp,
                    bias=bhd[32 : 32 + MPC, :],
                )
                nc.sync.dma_start(mu_d[:, c0_t : c0_t + CH], mu_t[:])
                nc.sync.dma_start(sig_d[:, c0_t : c0_t + CH], sig_t[:])

            for u, (nt, bi) in enumerate(units):
                c0 = nt * CH
                grp = groups[bi]
                # make sure this unit's layer-1 has been emitted (normally a
                # no-op: the items drained via pop_side during the prior unit)
                emit_l1_now(nt, bi)
                hcur = {m: h_l1.pop((nt, m)) for m in grp}

                def rhs(m, s):
                    t = hcur[m]
                    if isinstance(t, tuple):
                        return t[0][:, t[1], s : s + MM_N]
                    return t[:, s : s + MM_N]

                def enqueue_heads(m, hcur=None, nt=nt, bi=bi, grp=grp):
                    def emit(
                        t=hcur[m],
                        m=m,
                        st=(m == grp[0]),
                        sp=(m == grp[-1]),
                    ):
                        if st:
                            grp_hp[(nt, bi)] = mmpsum.tile(
                                [64, CH], f32, tag="mm", name="hpg"
                            )
                        hpg = grp_hp[(nt, bi)]
                        for s in range(0, CH, MM_N):
                            nc.tensor.matmul(
                                hpg[:, s : s + MM_N],
                                whd[:, m * 64 : (m + 1) * 64],
                                t[:, s : s + MM_N],
                                start=st,
                                stop=sp,
                            )

                    side.append((("hd", nt), emit))
                    if m == grp[-1]:
                        def drain(nt=nt, bi=bi):
                            hpg = grp_hp.pop((nt, bi))
                            acc = accpool.tile([64, CH], f32, tag="acc")
                            if bi == 0:
                                nc.vector.tensor_scalar(
                                    acc[:], hpg[:], 0.0, 0.0, ALU.add
                                )
                            else:
                                nc.vector.tensor_tensor(
                                    acc[:], hpg[:], chunk_acc[nt][:], ALU.add
                                )
                            chunk_acc[nt] = acc

                        eng_t["dve"] += cost("dve", CH)
                        side.append((("hd", nt), drain))

                # hidden layers, interleaved across the group
                for i in range(NH):
                    for m in grp:
                        ps = mmpsum.tile([128, CH], f32, tag="mm")
                        lhsh = wht[:, (m * NH + i) * H : (m * NH + i + 1) * H]
                        for s in range(0, CH, MM_N):
                            nc.tensor.matmul(
                                ps[:, s : s + MM_N],
                                lhsh,
                                rhs(m, s),
                                start=True,
                                stop=True,
                            )
                        hn = hpool.tile([128, CH], f16, tag="h")
                        relu(hn[:], ps[:], bias_ap=bh[:, m * NH + i : m * NH + i + 1])
                        hcur[m] = hn
                        pop_side()
                        if i == NH - 1:
                            enqueue_heads(m, hcur=hcur)
                    if i == 0 and pending_ep:
                        # previous chunk's mu/sigma finish; its head matmuls
                        # must be fully emitted first (in-order ACT queue)
                        c0_t, nt_t = pending_ep.pop()
                        flush_side(lambda t: t == ("hd", nt_t))
                        emit_epilogue(chunk_acc.pop(nt_t), c0_t)
                    if i == 1 and u + 1 < len(units):
                        # prefetch next unit's layer-1, spread via the queue
                        enqueue_l1(*units[u + 1])
                if bi == len(groups) - 1:
                    pending_ep.append((c0, nt))
            while pending_ep:
                c0_t, nt_t = pending_ep.pop()
                flush_side(lambda t: t == ("hd", nt_t))
                emit_epilogue(chunk_acc.pop(nt_t), c0_t)

    nc.compile()
    return nc


def _get_module():
    if "nc" not in _CACHE:
        _CACHE["nc"] = _build_module()
    return _CACHE["nc"]


def _shard_inputs(x, W1, b1, Wh, bh, Wmu, bmu, Wsig, bsig):
    """Build the per-core input maps (host-side layout prep)."""
    NBLK = (MPC + 3) // 4
    in_maps = []
    for c in range(NCORES):
        mb, half = c % NB, c // NB
        ms = slice(MPC * mb, MPC * (mb + 1))
        xh = x[NHALF * half : NHALF * (half + 1), :]  # [8192, 16]
        xtr = np.ascontiguousarray(xh.T)  # [16, 8192]
        xt_full = np.zeros((128, NHALF), dtype=np.float16)
        for rep in range(4):  # replicas at partition 0/32/64/96 for row tiling
            xt_full[32 * rep : 32 * rep + D, :] = xtr
            xt_full[32 * rep + D, :] = 1.0  # ones-row: bias via matmul

        w1 = W1[ms]  # [25, 128, 16]
        b1c = b1[ms]  # [25, 128]
        w1t = np.zeros((128, NBLK * H), dtype=np.float16)
        for m in range(MPC):
            b, g = m // 4, m % 4
            w1t[32 * g : 32 * g + D, b * H : (b + 1) * H] = w1[m].T
            w1t[32 * g + D, b * H : (b + 1) * H] = b1c[m]

        wh = Wh[ms]  # [25, 4, 128, 128] (out, in)
        wht = np.ascontiguousarray(
            wh.transpose(3, 0, 1, 2).reshape(H, MPC * NH * H)
        ).astype(np.float16)  # [h_in, (m, i, h_out)]

        whd = np.zeros((H, MPC * 64), dtype=np.float16)
        for m in range(MPC):
            base = m * 64
            whd[:, base + m] = Wmu[ms][m, 0, :]
            whd[:, base + 32 + m] = Wsig[ms][m, 0, :]

        bhp = np.ascontiguousarray(
            bh[ms].transpose(2, 0, 1).reshape(H, MPC * NH)
        )  # [128, (m, i)]
        bhdp = np.zeros((64, 1), dtype=np.float32)
        bhdp[0:MPC, 0] = bmu[ms][:, 0]
        bhdp[32 : 32 + MPC, 0] = bsig[ms][:, 0]

        in_maps.append(
            {
                "xt": xt_full,
                "w1t": w1t,
                "wht": wht,
                "whd": whd,
                "bh": bhp,
                "bhd": bhdp,
            }
        )
    return in_maps


def _run(in_maps, trace=False):
    from concourse.bass_utils import run_bass_kernel_spmd

    nc = _get_module()
    return run_bass_kernel_spmd(
        nc, in_maps, list(range(NCORES)), trace=trace
    )


def kernel(x, W1, b1, Wh, bh, Wmu, bmu, Wsig, bsig):
    args = [
        np.ascontiguousarray(np.asarray(a, dtype=np.float32))
        for a in (x, W1, b1, Wh, bh, Wmu, bmu, Wsig, bsig)
    ]
    in_maps = _shard_inputs(*args)
    res = _run(in_maps, trace=bool(int(os.environ.get("KERNEL_TRACE", "0"))))
    _CACHE["last_results"] = res

    mu = np.empty((M, N), dtype=np.float32)
    sig = np.empty((M, N), dtype=np.float32)
    for c in range(NCORES):
        mb, half = c % NB, c // NB
        m0 = MPC * mb
        ns = slice(NHALF * half, NHALF * (half + 1))
        r = res.results[c]
        mu[m0 : m0 + MPC, ns] = r["mu"]
        sig[m0 : m0 + MPC, ns] = r["sig"]
    return (mu.reshape(M, N, O), sig.reshape(M, N, O))


# revision 15
# speedup vs baseline: 1.3473x; 1.1224x over previous
"""Bootstrap-ensemble MLP (100 models, D=16 -> H=128 x5 -> mu/sigma heads)
on 8 Trainium2 NeuronCores.

Sharding: every core runs an identical SPMD program over 25 models x 8192
batch points (model axis split 4 ways x batch split 2 ways).  All per-core
weights are pre-arranged on the host into the exact SBUF layouts the
TensorEngine wants (lhsT = pre-transposed stationary operand).

Key perf structure:
- fp16 matmuls (1 cycle/column on the PE vs ~2.2 for fp32r), fp32 PSUM
  accumulate + fp32 bias -> accuracy stays ~1e-3.
- layer-1 bias folded into the matmul via a ones-row (K=17): the L1
  ReLU needs no per-model bias operand, so one fused op covers the two
  models sharing a PSUM tile.
- bias+ReLU ops are greedily balanced across ACT (1.2 GHz) and DVE
  (0.96 GHz) using measured per-op costs (only those two engines can
  read PSUM).
- mu/sigma head matmuls accumulate all 25 models into one [64, CH] PSUM
  tile via zero-padded per-model head weights; finished with
  Identity/Exp activations with the bias folded in.
"""

import os

import numpy as np

M = 100  # n_models
D = 16  # input_dim
H = 128  # hidden_dim
O = 1  # output_dim
NH = 4  # n_hidden
N = 16384  # batch of query points

NCORES = 8
MPC = 25  # models per core
NB = 4  # model blocks
NHALF = N // 2  # 8192 points per core
CH = 1024  # chunk of batch points processed at once
NCH = NHALF // CH  # 8 chunks
MM_N = 512  # matmul moving free dim (one PSUM bank of fp32)
KL1 = D + 1  # L1 contraction rows: 16 inputs + 1 ones-row for the bias

_CACHE: dict = {}


def _build_module():
    import concourse.bacc as bacc
    import concourse.mybir as mybir
    import concourse.tile as tile

    f32 = mybir.dt.float32
    f16 = mybir.dt.float16
    AF = mybir.ActivationFunctionType
    ALU = mybir.AluOpType

    nc = bacc.Bacc(
        "TRN2",
        target_bir_lowering=False,
        debug=False,
        num_devices=NCORES,
    )

    NBLK = (MPC + 3) // 4  # 7 row-tiling blocks of up to 4 models
    xt_d = nc.dram_tensor("xt", [128, NHALF], f16, kind="ExternalInput")
    w1t_d = nc.dram_tensor("w1t", [128, NBLK * H], f16, kind="ExternalInput")
    wht_d = nc.dram_tensor("wht", [H, MPC * NH * H], f16, kind="ExternalInput")
    whd_d = nc.dram_tensor("whd", [H, MPC * 64], f16, kind="ExternalInput")
    bh_d = nc.dram_tensor("bh", [H, MPC * NH], f32, kind="ExternalInput")
    bhd_d = nc.dram_tensor("bhd", [64, 1], f32, kind="ExternalInput")
    mu_d = nc.dram_tensor("mu", [MPC, NHALF], f32, kind="ExternalOutput")
    sig_d = nc.dram_tensor("sig", [MPC, NHALF], f32, kind="ExternalOutput")

    with tile.TileContext(nc) as tc:
        with (
            tc.tile_pool(name="const", bufs=1) as const,
            tc.tile_pool(name="ppool", bufs=8) as ppool,
            tc.tile_pool(name="hpool", bufs=16) as hpool,
            tc.tile_pool(name="opool", bufs=4) as opool,
            tc.tile_pool(name="accpool", bufs=3) as accpool,
            tc.tile_pool(name="mmpsum", bufs=4, space="PSUM") as mmpsum,
        ):
            xt = const.tile([128, NHALF], f16)
            w1t = const.tile([128, NBLK * H], f16)
            wht = const.tile([H, MPC * NH * H], f16)
            whd = const.tile([H, MPC * 64], f16)
            bh = const.tile([H, MPC * NH], f32)
            bhd = const.tile([64, 1], f32)

            nc.sync.dma_start(w1t[:], w1t_d[:])
            nc.sync.dma_start(bh[:], bh_d[:])
            nc.sync.dma_start(bhd[:], bhd_d[:])
            nc.sync.dma_start(whd[:], whd_d[:])
            # chunked so the first models' matmuls don't wait on the full blob
            for m in range(MPC):
                s = m * NH * H
                nc.sync.dma_start(wht[:, s : s + NH * H], wht_d[:, s : s + NH * H])
            for nt in range(NCH):
                s = nt * CH
                nc.sync.dma_start(xt[:, s : s + CH], xt_d[:, s : s + CH])

            # --- greedy ACT/DVE balance (measured per-op cost, ns) ---
            eng_t = {"act": 0.0, "dve": 0.0}

            def cost(eng, cols):
                return cols * 0.836 + 257.0 if eng == "act" else cols * 1.035 + 215.0

            def relu(dst, src, bias_ap=None, cols=CH, pin=None):
                e = pin or min(eng_t, key=lambda k: eng_t[k] + cost(k, cols))
                eng_t[e] += cost(e, cols)
                if e == "act":
                    nc.scalar.activation(
                        dst, src, AF.Relu,
                        bias=bias_ap if bias_ap is not None else 0.0,
                    )
                else:
                    nc.vector.tensor_scalar(
                        dst, src, bias_ap if bias_ap is not None else 0.0,
                        0.0, ALU.add, ALU.max,
                    )

            # wide interleave: 8-9 parallel model-chains per unit so the two
            # ReLU engines always have ready work despite in-order queues
            groups = [list(range(0, 8)), list(range(8, 16)), list(range(16, MPC))]
            units = [(nt, bi) for nt in range(NCH) for bi in range(len(groups))]
            h_l1 = {}

            # side-work queue: L1-prefetch and head matmuls are emitted one
            # item at a time between hidden-layer slots, so the PE's hidden
            # PSUM-tile stream (which feeds ACT/DVE) never pauses for a burst
            from collections import deque

            side = deque()  # items: (tag, closure)

            def pop_side(n=1):
                for _ in range(n):
                    if side:
                        side.popleft()[1]()

            def flush_side(pred):
                keep = deque()
                while side:
                    tag, fn = side.popleft()
                    if pred(tag):
                        fn()
                    else:
                        keep.append((tag, fn))
                side.extend(keep)

            def emit_l1_pair(nt, m0, s):
                # L1 for models m0, m0+1 (same w1t 4-model block; quadrants
                # m%4): bias rides the ones-row, so one fused bias-free ReLU
                c0 = nt * CH
                nm = min(2, MPC - m0)
                if s == 0 and (nt, m0) not in h_l1:
                    pt = ppool.tile([128, 2, CH], f16, tag="hp2", name="hpair")
                    for jj in range(nm):
                        h_l1[(nt, m0 + jj)] = (pt, jj)
                pt = h_l1[(nt, m0)][0]
                ps = mmpsum.tile([128, 2, MM_N], f32, tag="mm", name="l1ps")
                for jj in range(nm):
                    m = m0 + jj
                    b, j = m // 4, m % 4
                    nc.tensor.matmul(
                        ps[:, jj, :],
                        w1t[32 * j : 32 * j + KL1, b * H : (b + 1) * H],
                        xt[32 * j : 32 * j + KL1, c0 + s : c0 + s + MM_N],
                        start=True,
                        stop=True,
                        tile_position=(32 * j, 0),
                    )
                relu(pt[:, 0:nm, s : s + MM_N], ps[:, 0:nm, :], cols=nm * MM_N)

            def enqueue_l1(nt, bi):
                grp = groups[bi]
                for s in range(0, CH, MM_N):
                    for m0 in range(grp[0], grp[-1] + 1, 2):
                        side.append(
                            (("l1", nt, bi), lambda m0=m0, s=s: emit_l1_pair(nt, m0, s))
                        )

            def emit_l1_now(nt, bi):
                flush_side(lambda t: t == ("l1", nt, bi))

            enqueue_l1(*units[0])
            emit_l1_now(*units[0])
            pending_ep = []
            grp_hp = {}  # (nt, bi) -> per-group head psum partial
            chunk_acc = {}  # nt -> running SBUF head accumulator

            def emit_epilogue(acc_t, c0_t):
                mu_t = opool.tile([MPC, CH], f32, tag="mu")
                sig_t = opool.tile([MPC, CH], f32, tag="sig")
                eng_t["dve"] += cost("dve", CH)
                nc.vector.tensor_scalar(
                    mu_t[:], acc_t[0:MPC, :], bhd[0:MPC, :], 0.0, ALU.add
                )
                eng_t["act"] += cost("act", CH)
                nc.scalar.activation(
                    sig_t[:], acc_t[32 : 32 + MPC, :], AF.Exp,
                    bias=bhd[32 : 32 + MPC, :],
                )
                nc.sync.dma_start(mu_d[:, c0_t : c0_t + CH], mu_t[:])
                nc.sync.dma_start(sig_d[:, c0_t : c0_t + CH], sig_t[:])

            for u, (nt, bi) in enumerate(units):
                c0 = nt * CH
                grp = groups[bi]
                # make sure this unit's layer-1 has been emitted (normally a
                # no-op: the items drained via pop_side during the prior unit)
                emit_l1_now(nt, bi)
                hcur = {m: h_l1.pop((nt, m)) for m in grp}

                def rhs(m, s):
                    t = hcur[m]
                    if isinstance(t, tuple):
                        return t[0][:, t[1], s : s + MM_N]
                    return t[:, s : s + MM_N]

                def enqueue_heads(m, hcur=None, nt=nt, bi=bi, grp=grp):
                    def emit(
                        t=hcur[m],
                        m=m,
                        st=(m == grp[0]),
                        sp=(m == grp[-1]),
                    ):
                        if st:
                            grp_hp[(nt, bi)] = mmpsum.tile(
                                [64, CH], f32, tag="mm", name="hpg"
                            )
                        hpg = grp_hp[(nt, bi)]
                        for s in range(0, CH, MM_N):
                            nc.tensor.matmul(
                                hpg[:, s : s + MM_N],
                                whd[:, m * 64 : (m + 1) * 64],
                                t[:, s : s + MM_N],
                                start=st,
                                stop=sp,
                            )

                    side.append((("hd", nt), emit))
                    if m == grp[-1]:
                        def drain(nt=nt, bi=bi):
                            hpg = grp_hp.pop((nt, bi))
                            acc = accpool.tile([64, CH], f32, tag="acc")
                            if bi == 0:
                                nc.vector.tensor_scalar(
                                    acc[:], hpg[:], 0.0, 0.0, ALU.add
                                )
                            else:
                                nc.vector.tensor_tensor(
                                    acc[:], hpg[:], chunk_acc[nt][:], ALU.add
                                )
                            chunk_acc[nt] = acc

                        eng_t["dve"] += cost("dve", CH)
                        side.append((("hd", nt), drain))

                # hidden layers, interleaved across the group
                for i in range(NH):
                    for m in grp:
                        ps = mmpsum.tile([128, CH], f32, tag="mm")
                        lhsh = wht[:, (m * NH + i) * H : (m * NH + i + 1) * H]
                        for s in range(0, CH, MM_N):
                            nc.tensor.matmul(
                                ps[:, s : s + MM_N],
                                lhsh,
                                rhs(m, s),
                                start=True,
                                stop=True,
                            )
                        hn = hpool.tile([128, CH], f16, tag="h")
                        relu(hn[:], ps[:], bias_ap=bh[:, m * NH + i : m * NH + i + 1])
                        hcur[m] = hn
                        pop_side()
                        if i == NH - 1:
                            enqueue_heads(m, hcur=hcur)
                    if i == 0 and pending_ep:
                        # previous chunk's mu/sigma finish; its head matmuls
                        # must be fully emitted first (in-order ACT queue)
                        c0_t, nt_t = pending_ep.pop()
                        flush_side(lambda t: t == ("hd", nt_t))
                        emit_epilogue(chunk_acc.pop(nt_t), c0_t)
                    if i == 1 and u + 1 < len(units):
                        # prefetch next unit's layer-1, spread via the queue
                        enqueue_l1(*units[u + 1])
                if bi == len(groups) - 1:
                    pending_ep.append((c0, nt))
            while pending_ep:
                c0_t, nt_t = pending_ep.pop()
                flush_side(lambda t: t == ("hd", nt_t))
                emit_epilogue(chunk_acc.pop(nt_t), c0_t)

    nc.compile()
    return nc


def _get_module():
    if "nc" not in _CACHE:
        _CACHE["nc"] = _build_module()
    return _CACHE["nc"]


def _shard_inputs(x, W1, b1, Wh, bh, Wmu, bmu, Wsig, bsig):
    """Build the per-core input maps (host-side layout prep)."""
    NBLK = (MPC + 3) // 4
    in_maps = []
    for c in range(NCORES):
        mb, half = c % NB, c // NB
        ms = slice(MPC * mb, MPC * (mb + 1))
        xh = x[NHALF * half : NHALF * (half + 1), :]  # [8192, 16]
        xtr = np.ascontiguousarray(xh.T)  # [16, 8192]
        xt_full = np.zeros((128, NHALF), dtype=np.float16)
        for rep in range(4):  # replicas at partition 0/32/64/96 for row tiling
            xt_full[32 * rep : 32 * rep + D, :] = xtr
            xt_full[32 * rep + D, :] = 1.0  # ones-row: bias via matmul

        w1 = W1[ms]  # [25, 128, 16]
        b1c = b1[ms]  # [25, 128]
        w1t = np.zeros((128, NBLK * H), dtype=np.float16)
        for m in range(MPC):
            b, g = m // 4, m % 4
            w1t[32 * g : 32 * g + D, b * H : (b + 1) * H] = w1[m].T
            w1t[32 * g + D, b * H : (b + 1) * H] = b1c[m]

        wh = Wh[ms]  # [25, 4, 128, 128] (out, in)
        wht = np.ascontiguousarray(
            wh.transpose(3, 0, 1, 2).reshape(H, MPC * NH * H)
        ).astype(np.float16)  # [h_in, (m, i, h_out)]

        whd = np.zeros((H, MPC * 64), dtype=np.float16)
        for m in range(MPC):
            base = m * 64
            whd[:, base + m] = Wmu[ms][m, 0, :]
            whd[:, base + 32 + m] = Wsig[ms][m, 0, :]

        bhp = np.ascontiguousarray(
            bh[ms].transpose(2, 0, 1).reshape(H, MPC * NH)
        )  # [128, (m, i)]
        bhdp = np.zeros((64, 1), dtype=np.float32)
        bhdp[0:MPC, 0] = bmu[ms][:, 0]
        bhdp[32 : 32 + MPC, 0] = bsig[ms][:, 0]

        in_maps.append(
            {
                "xt": xt_full,
                "w1t": w1t,
                "wht": wht,
                "whd": whd,
                "bh": bhp,
                "bhd": bhdp,
            }
        )
    return in_maps


def _run(in_maps, trace=False):
    from concourse.bass_utils import run_bass_kernel_spmd

    nc = _get_module()
    return run_bass_kernel_spmd(
        nc, in_maps, list(range(NCORES)), trace=trace
    )


def kernel(x, W1, b1, Wh, bh, Wmu, bmu, Wsig, bsig):
    args = [
        np.ascontiguousarray(np.asarray(a, dtype=np.float32))
        for a in (x, W1, b1, Wh, bh, Wmu, bmu, Wsig, bsig)
    ]
    in_maps = _shard_inputs(*args)
    res = _run(in_maps, trace=bool(int(os.environ.get("KERNEL_TRACE", "0"))))
    _CACHE["last_results"] = res

    mu = np.empty((M, N), dtype=np.float32)
    sig = np.empty((M, N), dtype=np.float32)
    for c in range(NCORES):
        mb, half = c % NB, c // NB
        m0 = MPC * mb
        ns = slice(NHALF * half, NHALF * (half + 1))
        r = res.results[c]
        mu[m0 : m0 + MPC, ns] = r["mu"]
        sig[m0 : m0 + MPC, ns] = r["sig"]
    return (mu.reshape(M, N, O), sig.reshape(M, N, O))


# revision 18
# speedup vs baseline: 1.3829x; 1.0264x over previous
"""Bootstrap-ensemble MLP (100 models, D=16 -> H=128 x5 -> mu/sigma heads)
on 8 Trainium2 NeuronCores.

Sharding: every core runs an identical SPMD program over 25 models x 8192
batch points (model axis split 4 ways x batch split 2 ways).  All per-core
weights are pre-arranged on the host into the exact SBUF layouts the
TensorEngine wants (lhsT = pre-transposed stationary operand).

Key perf structure:
- fp16 matmuls (1 cycle/column on the PE vs ~2.2 for fp32r), fp32 PSUM
  accumulate + fp32 bias -> accuracy stays ~1e-3.
- layer-1 bias folded into the matmul via a ones-row (K=17): the L1
  ReLU needs no per-model bias operand, so one fused op covers the two
  models sharing a PSUM tile.
- bias+ReLU ops are greedily balanced across ACT (1.2 GHz) and DVE
  (0.96 GHz) using measured per-op costs (only those two engines can
  read PSUM).
- mu/sigma head matmuls accumulate all 25 models into one [64, CH] PSUM
  tile via zero-padded per-model head weights; finished with
  Identity/Exp activations with the bias folded in.
"""

import os

import numpy as np

M = 100  # n_models
D = 16  # input_dim
H = 128  # hidden_dim
O = 1  # output_dim
NH = 4  # n_hidden
N = 16384  # batch of query points

NCORES = 8
MPC = 25  # models per core
NB = 4  # model blocks
NHALF = N // 2  # 8192 points per core
CH = 1024  # chunk of batch points processed at once
NCH = NHALF // CH  # 8 chunks
MM_N = 512  # matmul moving free dim (one PSUM bank of fp32)
KL1 = D + 1  # L1 contraction rows: 16 inputs + 1 ones-row for the bias

_CACHE: dict = {}


def _build_module():
    import concourse.bacc as bacc
    import concourse.mybir as mybir
    import concourse.tile as tile

    f32 = mybir.dt.float32
    f16 = mybir.dt.float16
    AF = mybir.ActivationFunctionType
    ALU = mybir.AluOpType

    nc = bacc.Bacc(
        "TRN2",
        target_bir_lowering=False,
        debug=False,
        num_devices=NCORES,
    )

    NBLK = (MPC + 3) // 4  # 7 row-tiling blocks of up to 4 models
    xt_d = nc.dram_tensor("xt", [128, NHALF], f16, kind="ExternalInput")
    w1t_d = nc.dram_tensor("w1t", [128, NBLK * H], f16, kind="ExternalInput")
    wht_d = nc.dram_tensor("wht", [H, MPC * NH * H], f16, kind="ExternalInput")
    whd_d = nc.dram_tensor("whd", [H, MPC * 64], f16, kind="ExternalInput")
    bh_d = nc.dram_tensor("bh", [H, MPC * NH], f32, kind="ExternalInput")
    bhd_d = nc.dram_tensor("bhd", [64, 1], f32, kind="ExternalInput")
    mu_d = nc.dram_tensor("mu", [MPC, NHALF], f32, kind="ExternalOutput")
    sig_d = nc.dram_tensor("sig", [MPC, NHALF], f32, kind="ExternalOutput")

    with tile.TileContext(nc) as tc:
        with (
            tc.tile_pool(name="const", bufs=1) as const,
            tc.tile_pool(name="ppool", bufs=10) as ppool,
            tc.tile_pool(name="hpool", bufs=20) as hpool,
            tc.tile_pool(name="opool", bufs=4) as opool,
            tc.tile_pool(name="accpool", bufs=3) as accpool,
            tc.tile_pool(name="mmpsum", bufs=4, space="PSUM") as mmpsum,
        ):
            xt = const.tile([128, NHALF], f16)
            w1t = const.tile([128, NBLK * H], f16)
            wht = const.tile([H, MPC * NH * H], f16)
            whd = const.tile([H, MPC * 64], f16)
            bh = const.tile([H, MPC * NH], f32)
            bhd = const.tile([64, 1], f32)

            # priority order: what the first unit's matmuls need comes first
            nc.sync.dma_start(w1t[:], w1t_d[:])
            nc.sync.dma_start(xt[:, 0:CH], xt_d[:, 0:CH])
            nc.sync.dma_start(bh[:], bh_d[:])
            nc.sync.dma_start(bhd[:], bhd_d[:])
            nc.sync.dma_start(whd[:], whd_d[:])
            # chunked so the first models' matmuls don't wait on the full blob
            for m in range(MPC):
                s = m * NH * H
                nc.sync.dma_start(wht[:, s : s + NH * H], wht_d[:, s : s + NH * H])
            for nt in range(1, NCH):
                s = nt * CH
                nc.sync.dma_start(xt[:, s : s + CH], xt_d[:, s : s + CH])

            # --- greedy ACT/DVE balance (measured per-op cost, ns) ---
            eng_t = {"act": 0.0, "dve": 0.0}

            def cost(eng, cols):
                return cols * 0.836 + 257.0 if eng == "act" else cols * 1.035 + 215.0

            def relu(dst, src, bias_ap=None, cols=CH, pin=None):
                e = pin or min(eng_t, key=lambda k: eng_t[k] + cost(k, cols))
                eng_t[e] += cost(e, cols)
                if e == "act":
                    nc.scalar.activation(
                        dst, src, AF.Relu,
                        bias=bias_ap if bias_ap is not None else 0.0,
                    )
                else:
                    nc.vector.tensor_scalar(
                        dst, src, bias_ap if bias_ap is not None else 0.0,
                        0.0, ALU.add, ALU.max,
                    )

            # wide interleave: 8-9 parallel model-chains per unit so the two
            # ReLU engines always have ready work despite in-order queues
            groups = [list(range(0, 8)), list(range(8, 16)), list(range(16, MPC))]
            units = [(nt, bi) for nt in range(NCH) for bi in range(len(groups))]
            h_l1 = {}

            # side-work queue: L1-prefetch and head matmuls are emitted one
            # item at a time between hidden-layer slots, so the PE's hidden
            # PSUM-tile stream (which feeds ACT/DVE) never pauses for a burst
            from collections import deque

            side = deque()  # items: (tag, closure)

            def pop_side(n=1):
                for _ in range(n):
                    if side:
                        side.popleft()[1]()

            def flush_side(pred):
                keep = deque()
                while side:
                    tag, fn = side.popleft()
                    if pred(tag):
                        fn()
                    else:
                        keep.append((tag, fn))
                side.extend(keep)

            def emit_l1_pair(nt, m0, s):
                # L1 for models m0, m0+1 (same w1t 4-model block; quadrants
                # m%4): bias rides the ones-row, so one fused bias-free ReLU
                c0 = nt * CH
                nm = min(2, MPC - m0)
                if s == 0 and (nt, m0) not in h_l1:
                    pt = ppool.tile([128, 2, CH], f16, tag="hp2", name="hpair")
                    for jj in range(nm):
                        h_l1[(nt, m0 + jj)] = (pt, jj)
                pt = h_l1[(nt, m0)][0]
                ps = mmpsum.tile([128, 2, MM_N], f32, tag="mm", name="l1ps")
                for jj in range(nm):
                    m = m0 + jj
                    b, j = m // 4, m % 4
                    nc.tensor.matmul(
                        ps[:, jj, :],
                        w1t[32 * j : 32 * j + KL1, b * H : (b + 1) * H],
                        xt[32 * j : 32 * j + KL1, c0 + s : c0 + s + MM_N],
                        start=True,
                        stop=True,
                        tile_position=(32 * j, 0),
                    )
                relu(pt[:, 0:nm, s : s + MM_N], ps[:, 0:nm, :], cols=nm * MM_N)

            def enqueue_l1(nt, bi):
                grp = groups[bi]
                for s in range(0, CH, MM_N):
                    for m0 in range(grp[0], grp[-1] + 1, 2):
                        side.append(
                            (("l1", nt, bi), lambda m0=m0, s=s: emit_l1_pair(nt, m0, s))
                        )

            def emit_l1_now(nt, bi):
                flush_side(lambda t: t == ("l1", nt, bi))

            enqueue_l1(*units[0])
            emit_l1_now(*units[0])
            pending_ep = []
            grp_hp = {}  # (nt, bi) -> per-group head psum partial
            chunk_acc = {}  # nt -> running SBUF head accumulator

            def emit_epilogue(acc_t, c0_t):
                mu_t = opool.tile([MPC, CH], f32, tag="mu")
                sig_t = opool.tile([MPC, CH], f32, tag="sig")
                eng_t["dve"] += cost("dve", CH)
                nc.vector.tensor_scalar(
                    mu_t[:], acc_t[0:MPC, :], bhd[0:MPC, :], 0.0, ALU.add
                )
                eng_t["act"] += cost("act", CH)
                nc.scalar.activation(
                    sig_t[:], acc_t[32 : 32 + MPC, :], AF.Exp,
                    bias=bhd[32 : 32 + MPC, :],
                )
                nc.sync.dma_start(mu_d[:, c0_t : c0_t + CH], mu_t[:])
                nc.sync.dma_start(sig_d[:, c0_t : c0_t + CH], sig_t[:])

            for u, (nt, bi) in enumerate(units):
                c0 = nt * CH
                grp = groups[bi]
                # make sure this unit's layer-1 has been emitted (normally a
                # no-op: the items drained via pop_side during the prior unit)
                emit_l1_now(nt, bi)
                hcur = {m: h_l1.pop((nt, m)) for m in grp}

                def rhs(m, s):
                    t = hcur[m]
                    if isinstance(t, tuple):
                        return t[0][:, t[1], s : s + MM_N]
                    return t[:, s : s + MM_N]

                def enqueue_heads(m, hcur=None, nt=nt, bi=bi, grp=grp):
                    def emit(
                        t=hcur[m],
                        m=m,
                        st=(m == grp[0]),
                        sp=(m == grp[-1]),
                    ):
                        if st:
                            grp_hp[(nt, bi)] = mmpsum.tile(
                                [64, CH], f32, tag="mm", name="hpg"
                            )
                        hpg = grp_hp[(nt, bi)]
                        for s in range(0, CH, MM_N):
                            nc.tensor.matmul(
                                hpg[:, s : s + MM_N],
                                whd[:, m * 64 : (m + 1) * 64],
                                t[:, s : s + MM_N],
                                start=st,
                                stop=sp,
                            )

                    side.append((("hd", nt), emit))
                    if m == grp[-1]:
                        def drain(nt=nt, bi=bi):
                            hpg = grp_hp.pop((nt, bi))
                            acc = accpool.tile([64, CH], f32, tag="acc")
                            if bi == 0:
                                nc.vector.tensor_scalar(
                                    acc[:], hpg[:], 0.0, 0.0, ALU.add
                                )
                            else:
                                nc.vector.tensor_tensor(
                                    acc[:], hpg[:], chunk_acc[nt][:], ALU.add
                                )
                            chunk_acc[nt] = acc

                        eng_t["dve"] += cost("dve", CH)
                        side.append((("hd", nt), drain))

                # hidden layers, interleaved across the group
                for i in range(NH):
                    for m in grp:
                        ps = mmpsum.tile([128, CH], f32, tag="mm")
                        lhsh = wht[:, (m * NH + i) * H : (m * NH + i + 1) * H]
                        for s in range(0, CH, MM_N):
                            nc.tensor.matmul(
                                ps[:, s : s + MM_N],
                                lhsh,
                                rhs(m, s),
                                start=True,
                                stop=True,
                            )
                        hn = hpool.tile([128, CH], f16, tag="h")
                        relu(hn[:], ps[:], bias_ap=bh[:, m * NH + i : m * NH + i + 1])
                        hcur[m] = hn
                        pop_side()
                        if i == NH - 1:
                            enqueue_heads(m, hcur=hcur)
                    if i == 0 and pending_ep:
                        # previous chunk's mu/sigma finish; its head matmuls
                        # must be fully emitted first (in-order ACT queue)
                        c0_t, nt_t = pending_ep.pop()
                        flush_side(lambda t: t == ("hd", nt_t))
                        emit_epilogue(chunk_acc.pop(nt_t), c0_t)
                    if i == 1 and u + 1 < len(units):
                        # prefetch next unit's layer-1, spread via the queue
                        enqueue_l1(*units[u + 1])
                if bi == len(groups) - 1:
                    pending_ep.append((c0, nt))
            while pending_ep:
                c0_t, nt_t = pending_ep.pop()
                flush_side(lambda t: t == ("hd", nt_t))
                emit_epilogue(chunk_acc.pop(nt_t), c0_t)

    nc.compile()
    return nc


def _get_module():
    if "nc" not in _CACHE:
        _CACHE["nc"] = _build_module()
    return _CACHE["nc"]


def _shard_inputs(x, W1, b1, Wh, bh, Wmu, bmu, Wsig, bsig):
    """Build the per-core input maps (host-side layout prep)."""
    NBLK = (MPC + 3) // 4
    in_maps = []
    for c in range(NCORES):
        mb, half = c % NB, c // NB
        ms = slice(MPC * mb, MPC * (mb + 1))
        xh = x[NHALF * half : NHALF * (half + 1), :]  # [8192, 16]
        xtr = np.ascontiguousarray(xh.T)  # [16, 8192]
        xt_full = np.zeros((128, NHALF), dtype=np.float16)
        for rep in range(4):  # replicas at partition 0/32/64/96 for row tiling
            xt_full[32 * rep : 32 * rep + D, :] = xtr
            xt_full[32 * rep + D, :] = 1.0  # ones-row: bias via matmul

        w1 = W1[ms]  # [25, 128, 16]
        b1c = b1[ms]  # [25, 128]
        w1t = np.zeros((128, NBLK * H), dtype=np.float16)
        for m in range(MPC):
            b, g = m // 4, m % 4
            w1t[32 * g : 32 * g + D, b * H : (b + 1) * H] = w1[m].T
            w1t[32 * g + D, b * H : (b + 1) * H] = b1c[m]

        wh = Wh[ms]  # [25, 4, 128, 128] (out, in)
        wht = np.ascontiguousarray(
            wh.transpose(3, 0, 1, 2).reshape(H, MPC * NH * H)
        ).astype(np.float16)  # [h_in, (m, i, h_out)]

        whd = np.zeros((H, MPC * 64), dtype=np.float16)
        for m in range(MPC):
            base = m * 64
            whd[:, base + m] = Wmu[ms][m, 0, :]
            whd[:, base + 32 + m] = Wsig[ms][m, 0, :]

        bhp = np.ascontiguousarray(
            bh[ms].transpose(2, 0, 1).reshape(H, MPC * NH)
        )  # [128, (m, i)]
        bhdp = np.zeros((64, 1), dtype=np.float32)
        bhdp[0:MPC, 0] = bmu[ms][:, 0]
        bhdp[32 : 32 + MPC, 0] = bsig[ms][:, 0]

        in_maps.append(
            {
                "xt": xt_full,
                "w1t": w1t,
                "wht": wht,
                "whd": whd,
                "bh": bhp,
                "bhd": bhdp,
            }
        )
    return in_maps


def _run(in_maps, trace=False):
    from concourse.bass_utils import run_bass_kernel_spmd

    nc = _get_module()
    return run_bass_kernel_spmd(
        nc, in_maps, list(range(NCORES)), trace=trace
    )


def kernel(x, W1, b1, Wh, bh, Wmu, bmu, Wsig, bsig):
    args = [
        np.ascontiguousarray(np.asarray(a, dtype=np.float32))
        for a in (x, W1, b1, Wh, bh, Wmu, bmu, Wsig, bsig)
    ]
    in_maps = _shard_inputs(*args)
    res = _run(in_maps, trace=bool(int(os.environ.get("KERNEL_TRACE", "0"))))
    _CACHE["last_results"] = res

    mu = np.empty((M, N), dtype=np.float32)
    sig = np.empty((M, N), dtype=np.float32)
    for c in range(NCORES):
        mb, half = c % NB, c // NB
        m0 = MPC * mb
        ns = slice(NHALF * half, NHALF * (half + 1))
        r = res.results[c]
        mu[m0 : m0 + MPC, ns] = r["mu"]
        sig[m0 : m0 + MPC, ns] = r["sig"]
    return (mu.reshape(M, N, O), sig.reshape(M, N, O))


# revision 20
# speedup vs baseline: 1.3996x; 1.0120x over previous
"""Bootstrap-ensemble MLP (100 models, D=16 -> H=128 x5 -> mu/sigma heads)
on 8 Trainium2 NeuronCores.

Sharding: every core runs an identical SPMD program over 25 models x 8192
batch points (model axis split 4 ways x batch split 2 ways).  All per-core
weights are pre-arranged on the host into the exact SBUF layouts the
TensorEngine wants (lhsT = pre-transposed stationary operand).

Key perf structure:
- fp16 matmuls (1 cycle/column on the PE vs ~2.2 for fp32r), fp32 PSUM
  accumulate + fp32 bias -> accuracy stays ~1e-3.
- layer-1 bias folded into the matmul via a ones-row (K=17): the L1
  ReLU needs no per-model bias operand, so one fused op covers the two
  models sharing a PSUM tile.
- bias+ReLU ops are greedily balanced across ACT (1.2 GHz) and DVE
  (0.96 GHz) using measured per-op costs (only those two engines can
  read PSUM).
- mu/sigma head matmuls accumulate all 25 models into one [64, CH] PSUM
  tile via zero-padded per-model head weights; finished with
  Identity/Exp activations with the bias folded in.
"""

import os

import numpy as np

M = 100  # n_models
D = 16  # input_dim
H = 128  # hidden_dim
O = 1  # output_dim
NH = 4  # n_hidden
N = 16384  # batch of query points

NCORES = 8
MPC = 25  # models per core
NB = 4  # model blocks
NHALF = N // 2  # 8192 points per core
CH = 1024  # chunk of batch points processed at once
NCH = NHALF // CH  # 8 chunks
MM_N = 512  # matmul moving free dim (one PSUM bank of fp32)
KL1 = D + 1  # L1 contraction rows: 16 inputs + 1 ones-row for the bias

_CACHE: dict = {}


def _build_module():
    import concourse.bacc as bacc
    import concourse.mybir as mybir
    import concourse.tile as tile

    f32 = mybir.dt.float32
    f16 = mybir.dt.float16
    AF = mybir.ActivationFunctionType
    ALU = mybir.AluOpType

    nc = bacc.Bacc(
        "TRN2",
        target_bir_lowering=False,
        debug=False,
        num_devices=NCORES,
    )

    NBLK = (MPC + 3) // 4  # 7 row-tiling blocks of up to 4 models
    xt_d = nc.dram_tensor("xt", [128, NHALF], f16, kind="ExternalInput")
    w1t_d = nc.dram_tensor("w1t", [128, NBLK * H], f16, kind="ExternalInput")
    wht_d = nc.dram_tensor("wht", [H, MPC * NH * H], f16, kind="ExternalInput")
    whd_d = nc.dram_tensor("whd", [H, MPC * 64], f16, kind="ExternalInput")
    bh_d = nc.dram_tensor("bh", [H, MPC * NH], f32, kind="ExternalInput")
    bhd_d = nc.dram_tensor("bhd", [64, 1], f32, kind="ExternalInput")
    mu_d = nc.dram_tensor("mu", [MPC, NHALF], f32, kind="ExternalOutput")
    sig_d = nc.dram_tensor("sig", [MPC, NHALF], f32, kind="ExternalOutput")

    with tile.TileContext(nc) as tc:
        with (
            tc.tile_pool(name="const", bufs=1) as const,
            tc.tile_pool(name="ppool", bufs=10) as ppool,
            tc.tile_pool(name="hpool", bufs=20) as hpool,
            tc.tile_pool(name="opool", bufs=4) as opool,
            tc.tile_pool(name="accpool", bufs=3) as accpool,
            tc.tile_pool(name="mmpsum", bufs=4, space="PSUM") as mmpsum,
        ):
            xt = const.tile([128, NHALF], f16)
            w1t = const.tile([128, NBLK * H], f16)
            wht = const.tile([H, MPC * NH * H], f16)
            whd = const.tile([H, MPC * 64], f16)
            bh = const.tile([H, MPC * NH], f32)
            bhd = const.tile([64, 1], f32)

            # priority order: what the first unit's matmuls need comes first
            nc.sync.dma_start(w1t[:], w1t_d[:])
            nc.sync.dma_start(xt[:, 0:CH], xt_d[:, 0:CH])
            nc.sync.dma_start(bh[:], bh_d[:])
            nc.sync.dma_start(bhd[:], bhd_d[:])
            nc.sync.dma_start(whd[:], whd_d[:])
            # chunked so the first models' matmuls don't wait on the full blob
            for m in range(MPC):
                s = m * NH * H
                nc.sync.dma_start(wht[:, s : s + NH * H], wht_d[:, s : s + NH * H])
            for nt in range(1, NCH):
                s = nt * CH
                nc.sync.dma_start(xt[:, s : s + CH], xt_d[:, s : s + CH])

            # --- greedy ACT/DVE balance (measured per-op cost, ns) ---
            eng_t = {"act": 0.0, "dve": 0.0}

            def cost(eng, cols):
                return cols * 0.836 + 257.0 if eng == "act" else cols * 1.035 + 215.0

            def relu(dst, src, bias_ap=None, cols=CH, pin=None):
                e = pin or min(eng_t, key=lambda k: eng_t[k] + cost(k, cols))
                eng_t[e] += cost(e, cols)
                if e == "act":
                    nc.scalar.activation(
                        dst, src, AF.Relu,
                        bias=bias_ap if bias_ap is not None else 0.0,
                    )
                else:
                    nc.vector.tensor_scalar(
                        dst, src, bias_ap if bias_ap is not None else 0.0,
                        0.0, ALU.add, ALU.max,
                    )

            # wide interleave: 12-13 parallel model-chains per unit so the two
            # ReLU engines always have ready work despite in-order queues
            groups = [list(range(0, 12)), list(range(12, MPC))]
            units = [(nt, bi) for nt in range(NCH) for bi in range(len(groups))]
            h_l1 = {}

            # side-work queue: L1-prefetch and head matmuls are emitted one
            # item at a time between hidden-layer slots, so the PE's hidden
            # PSUM-tile stream (which feeds ACT/DVE) never pauses for a burst
            from collections import deque

            side = deque()  # items: (tag, closure)

            def pop_side(n=1):
                for _ in range(n):
                    if side:
                        side.popleft()[1]()

            def flush_side(pred):
                keep = deque()
                while side:
                    tag, fn = side.popleft()
                    if pred(tag):
                        fn()
                    else:
                        keep.append((tag, fn))
                side.extend(keep)

            def emit_l1_pair(nt, m0, s):
                # L1 for models m0, m0+1 (same w1t 4-model block; quadrants
                # m%4): bias rides the ones-row, so one fused bias-free ReLU
                c0 = nt * CH
                nm = min(2, MPC - m0)
                if s == 0 and (nt, m0) not in h_l1:
                    pt = ppool.tile([128, 2, CH], f16, tag="hp2", name="hpair")
                    for jj in range(nm):
                        h_l1[(nt, m0 + jj)] = (pt, jj)
                pt = h_l1[(nt, m0)][0]
                ps = mmpsum.tile([128, 2, MM_N], f32, tag="mm", name="l1ps")
                for jj in range(nm):
                    m = m0 + jj
                    b, j = m // 4, m % 4
                    nc.tensor.matmul(
                        ps[:, jj, :],
                        w1t[32 * j : 32 * j + KL1, b * H : (b + 1) * H],
                        xt[32 * j : 32 * j + KL1, c0 + s : c0 + s + MM_N],
                        start=True,
                        stop=True,
                        tile_position=(32 * j, 0),
                    )
                relu(pt[:, 0:nm, s : s + MM_N], ps[:, 0:nm, :], cols=nm * MM_N)

            def enqueue_l1(nt, bi):
                grp = groups[bi]
                for s in range(0, CH, MM_N):
                    for m0 in range(grp[0], grp[-1] + 1, 2):
                        side.append(
                            (("l1", nt, bi), lambda m0=m0, s=s: emit_l1_pair(nt, m0, s))
                        )

            def emit_l1_now(nt, bi):
                flush_side(lambda t: t == ("l1", nt, bi))

            enqueue_l1(*units[0])
            emit_l1_now(*units[0])
            pending_ep = []
            grp_hp = {}  # (nt, bi) -> per-group head psum partial
            chunk_acc = {}  # nt -> running SBUF head accumulator

            def emit_epilogue(acc_t, c0_t):
                mu_t = opool.tile([MPC, CH], f32, tag="mu")
                sig_t = opool.tile([MPC, CH], f32, tag="sig")
                eng_t["dve"] += cost("dve", CH)
                nc.vector.tensor_scalar(
                    mu_t[:], acc_t[0:MPC, :], bhd[0:MPC, :], 0.0, ALU.add
                )
                eng_t["act"] += cost("act", CH)
                nc.scalar.activation(
                    sig_t[:], acc_t[32 : 32 + MPC, :], AF.Exp,
                    bias=bhd[32 : 32 + MPC, :],
                )
                nc.sync.dma_start(mu_d[:, c0_t : c0_t + CH], mu_t[:])
                nc.sync.dma_start(sig_d[:, c0_t : c0_t + CH], sig_t[:])

            for u, (nt, bi) in enumerate(units):
                c0 = nt * CH
                grp = groups[bi]
                # make sure this unit's layer-1 has been emitted (normally a
                # no-op: the items drained via pop_side during the prior unit)
                emit_l1_now(nt, bi)
                hcur = {m: h_l1.pop((nt, m)) for m in grp}

                def rhs(m, s):
                    t = hcur[m]
                    if isinstance(t, tuple):
                        return t[0][:, t[1], s : s + MM_N]
                    return t[:, s : s + MM_N]

                # heads accumulate per half-group (keeps the PSUM partial's
                # ring-slot residency short), then chain-add into SBUF on DVE
                half = len(grp) // 2
                segs = [grp[:half], grp[half:]]

                def enqueue_heads(m, hcur=None, nt=nt, bi=bi, segs=segs):
                    seg = segs[0] if m in segs[0] else segs[1]
                    si = 0 if m in segs[0] else 1
                    key = (nt, bi, si)

                    def emit(t=hcur[m], m=m, st=(m == seg[0]), sp=(m == seg[-1]), key=key):
                        if st:
                            grp_hp[key] = mmpsum.tile(
                                [64, CH], f32, tag="mm", name="hpg"
                            )
                        hpg = grp_hp[key]
                        for s in range(0, CH, MM_N):
                            nc.tensor.matmul(
                                hpg[:, s : s + MM_N],
                                whd[:, m * 64 : (m + 1) * 64],
                                t[:, s : s + MM_N],
                                start=st,
                                stop=sp,
                            )

                    side.append((("hd", nt), emit))
                    if m == seg[-1]:
                        def drain(nt=nt, key=key):
                            hpg = grp_hp.pop(key)
                            if nt not in chunk_acc:
                                acc = accpool.tile([64, CH], f32, tag="acc")
                                nc.vector.tensor_scalar(
                                    acc[:], hpg[:], 0.0, 0.0, ALU.add
                                )
                            else:
                                acc = accpool.tile([64, CH], f32, tag="acc")
                                nc.vector.tensor_tensor(
                                    acc[:], hpg[:], chunk_acc[nt][:], ALU.add
                                )
                            chunk_acc[nt] = acc

                        eng_t["dve"] += cost("dve", CH)
                        side.append((("hd", nt), drain))

                # hidden layers, interleaved across the group
                for i in range(NH):
                    for m in grp:
                        ps = mmpsum.tile([128, CH], f32, tag="mm")
                        lhsh = wht[:, (m * NH + i) * H : (m * NH + i + 1) * H]
                        for s in range(0, CH, MM_N):
                            nc.tensor.matmul(
                                ps[:, s : s + MM_N],
                                lhsh,
                                rhs(m, s),
                                start=True,
                                stop=True,
                            )
                        hn = hpool.tile([128, CH], f16, tag="h")
                        relu(hn[:], ps[:], bias_ap=bh[:, m * NH + i : m * NH + i + 1])
                        hcur[m] = hn
                        pop_side()
                        if i == NH - 1:
                            enqueue_heads(m, hcur=hcur)
                    if i == 0 and pending_ep:
                        # previous chunk's mu/sigma finish; its head matmuls
                        # must be fully emitted first (in-order ACT queue)
                        c0_t, nt_t = pending_ep.pop()
                        flush_side(lambda t: t == ("hd", nt_t))
                        emit_epilogue(chunk_acc.pop(nt_t), c0_t)
                    if i == 1 and u + 1 < len(units):
                        # prefetch next unit's layer-1, spread via the queue
                        enqueue_l1(*units[u + 1])
                if bi == len(groups) - 1:
                    pending_ep.append((c0, nt))
            while pending_ep:
                c0_t, nt_t = pending_ep.pop()
                flush_side(lambda t: t == ("hd", nt_t))
                emit_epilogue(chunk_acc.pop(nt_t), c0_t)

    nc.compile()
    return nc


def _get_module():
    if "nc" not in _CACHE:
        _CACHE["nc"] = _build_module()
    return _CACHE["nc"]


def _shard_inputs(x, W1, b1, Wh, bh, Wmu, bmu, Wsig, bsig):
    """Build the per-core input maps (host-side layout prep)."""
    NBLK = (MPC + 3) // 4
    in_maps = []
    for c in range(NCORES):
        mb, half = c % NB, c // NB
        ms = slice(MPC * mb, MPC * (mb + 1))
        xh = x[NHALF * half : NHALF * (half + 1), :]  # [8192, 16]
        xtr = np.ascontiguousarray(xh.T)  # [16, 8192]
        xt_full = np.zeros((128, NHALF), dtype=np.float16)
        for rep in range(4):  # replicas at partition 0/32/64/96 for row tiling
            xt_full[32 * rep : 32 * rep + D, :] = xtr
            xt_full[32 * rep + D, :] = 1.0  # ones-row: bias via matmul

        w1 = W1[ms]  # [25, 128, 16]
        b1c = b1[ms]  # [25, 128]
        w1t = np.zeros((128, NBLK * H), dtype=np.float16)
        for m in range(MPC):
            b, g = m // 4, m % 4
            w1t[32 * g : 32 * g + D, b * H : (b + 1) * H] = w1[m].T
            w1t[32 * g + D, b * H : (b + 1) * H] = b1c[m]

        wh = Wh[ms]  # [25, 4, 128, 128] (out, in)
        wht = np.ascontiguousarray(
            wh.transpose(3, 0, 1, 2).reshape(H, MPC * NH * H)
        ).astype(np.float16)  # [h_in, (m, i, h_out)]

        whd = np.zeros((H, MPC * 64), dtype=np.float16)
        for m in range(MPC):
            base = m * 64
            whd[:, base + m] = Wmu[ms][m, 0, :]
            whd[:, base + 32 + m] = Wsig[ms][m, 0, :]

        bhp = np.ascontiguousarray(
            bh[ms].transpose(2, 0, 1).reshape(H, MPC * NH)
        )  # [128, (m, i)]
        bhdp = np.zeros((64, 1), dtype=np.float32)
        bhdp[0:MPC, 0] = bmu[ms][:, 0]
        bhdp[32 : 32 + MPC, 0] = bsig[ms][:, 0]

        in_maps.append(
            {
                "xt": xt_full,
                "w1t": w1t,
                "wht": wht,
                "whd": whd,
                "bh": bhp,
                "bhd": bhdp,
            }
        )
    return in_maps


def _run(in_maps, trace=False):
    from concourse.bass_utils import run_bass_kernel_spmd

    nc = _get_module()
    return run_bass_kernel_spmd(
        nc, in_maps, list(range(NCORES)), trace=trace
    )


def kernel(x, W1, b1, Wh, bh, Wmu, bmu, Wsig, bsig):
    args = [
        np.ascontiguousarray(np.asarray(a, dtype=np.float32))
        for a in (x, W1, b1, Wh, bh, Wmu, bmu, Wsig, bsig)
    ]
    in_maps = _shard_inputs(*args)
    res = _run(in_maps, trace=bool(int(os.environ.get("KERNEL_TRACE", "0"))))
    _CACHE["last_results"] = res

    mu = np.empty((M, N), dtype=np.float32)
    sig = np.empty((M, N), dtype=np.float32)
    for c in range(NCORES):
        mb, half = c % NB, c // NB
        m0 = MPC * mb
        ns = slice(NHALF * half, NHALF * (half + 1))
        r = res.results[c]
        mu[m0 : m0 + MPC, ns] = r["mu"]
        sig[m0 : m0 + MPC, ns] = r["sig"]
    return (mu.reshape(M, N, O), sig.reshape(M, N, O))
